# revision 1
# baseline (speedup 1.0000x reference)
"""Trainium2 Bass kernel for the DSSM (dual-modality Mamba-style 2D selective
scan) module. 8-core SPMD: scan channels d-sharded (24/core x 4 directions),
upstream in_proj/dwconv d-sharded, downstream LN/out position-sharded.
Cross-core: one AllReduce (x_dbl partials + chan-attn MLP partials) and one
AllToAll (y halves channel->position reshard).
"""
import sys
sys.path.insert(0, "/opt/trn_rl_repo")
import numpy as np
import concourse.bass as bass
from concourse import mybir
from concourse.bacc import Bacc
from concourse.tile import TileContext
from concourse.bass_utils import run_bass_kernel_spmd

F32 = mybir.dt.float32
AF = mybir.ActivationFunctionType
OP = mybir.AluOpType

NCORES = 8
RG = [list(range(NCORES))]
B, H, W = 1, 48, 48
HW = H * W                      # 2304
L = 2 * HW                      # 4608
DM = 96                         # d_model
DI = 192                        # d_inner
NST = 4                         # d_state
RNK = 6                         # dt_rank
K = 4
DSL = DI // NCORES              # 24 channels per core
LANES = NST * DSL               # 96 scan lanes (lane = n*DSL + d)
CH = 512                        # phase-B column chunk
NCH = L // CH                   # 9
PC = HW // NCORES               # 288 positions per core (phase C)
RCH = 480                       # phase-A chunk = 10 image rows
ROWCHUNKS = [(0, 10), (10, 10), (20, 10), (30, 10), (40, 8)]
# r1 allreduce buffer: [14, 6*2304 + 4] (xdbl partials | attn v1 partials)
R1C = 6 * HW + 4
HALF_OFF = {(0, 0): 0, (0, 1): HW, (1, 0): 2 * HW, (1, 1): 3 * HW,
            (2, 1): 4 * HW, (3, 1): 5 * HW}  # (k, half)->col offset in r1

_cache = {}


def _build():
    nc = Bacc(trn_type="TRN2", num_devices=NCORES)
    EIn = dict(kind="ExternalInput")
    # per-core inputs (host-prepped)
    i_xvt = nc.dram_tensor("xvt", [DM, HW], F32, **EIn)
    i_xit = nc.dram_tensor("xit", [DM, HW], F32, **EIn)
    i_wxv = nc.dram_tensor("wxv", [DM, DSL], F32, **EIn)    # in_proj xv rows
    i_wzv = nc.dram_tensor("wzv", [DM, DSL], F32, **EIn)    # in_proj zv rows
    i_wxi = nc.dram_tensor("wxi", [DM, DSL], F32, **EIn)
    i_wzi = nc.dram_tensor("wzi", [DM, DSL], F32, **EIn)
    i_wsub = nc.dram_tensor("wsub", [DM, DSL], F32, **EIn)
    i_w9 = nc.dram_tensor("w9", [DSL, 3, 9, DSL], F32, **EIn)  # conv diag/tap/group
    i_b72 = nc.dram_tensor("b72", [DSL, 3], F32, **EIn)        # conv bias per group
    i_wpk = nc.dram_tensor("wpk", [DSL, K, 14], F32, **EIn)    # x_dbl partial lhsT
    i_wdtr = nc.dram_tensor("wdtr", [RNK, K, LANES], F32, **EIn)
    i_dtb = nc.dram_tensor("dtb", [LANES, K], F32, **EIn)
    i_asc = nc.dram_tensor("asc", [LANES, K], F32, **EIn)
    i_rep24 = nc.dram_tensor("rep24", [DSL, LANES], F32, **EIn)
    i_repb = nc.dram_tensor("repb", [NST, LANES], F32, **EIn)
    i_m96 = nc.dram_tensor("m96", [LANES, DSL], F32, **EIn)
    i_diagd = nc.dram_tensor("diagd", [DSL, 2, DSL], F32, **EIn)  # (vi,ir) summed D
    i_f1 = nc.dram_tensor("f1", [DSL, 4, 12], F32, **EIn)   # (via,vim,ira,irm)
    i_f2 = nc.dram_tensor("f2", [12, 2, 2, DM], F32, **EIn)  # (mod, chunk, out96)
    i_lnw = nc.dram_tensor("lnw", [DM, 2, 4], F32, **EIn)    # (chunk, gvi bvi gir bir)
    i_wout = nc.dram_tensor("wout", [DM, 2, DM], F32, **EIn)  # (contract chunk, out)
    i_wz = nc.dram_tensor("wz", [DM, 4, DM], F32, **EIn)     # z lhsT (vi0,vi1,ir0,ir1)
    i_onec = nc.dram_tensor("onec", [DM, 1], F32, **EIn)
    i_oner = nc.dram_tensor("oner", [1, DM], F32, **EIn)
    i_xvc = nc.dram_tensor("xvc", [DM, PC], F32, **EIn)
    i_xic = nc.dram_tensor("xic", [DM, PC], F32, **EIn)
    o_out = nc.dram_tensor("out", [DM, PC], F32, kind="ExternalOutput")
    # collective DRAM buffers
    d_r1i = nc.dram_tensor("d_r1i", [14, R1C], F32)
    d_r1o = nc.dram_tensor("d_r1o", [14, R1C], F32, addr_space="Shared")
    d_a2i = nc.dram_tensor("d_a2i", [NCORES, 2 * DSL, PC], F32)
    d_a2o = nc.dram_tensor("d_a2o", [NCORES, 2 * DSL, PC], F32)

    import contextlib
    with TileContext(nc) as tc, contextlib.ExitStack() as ctx:
        wpool = ctx.enter_context(tc.tile_pool(name="weights", bufs=1))
        big = ctx.enter_context(tc.tile_pool(name="big", bufs=1))

        # ---- load weights ----
        def wtile(shape, src):
            t = wpool.tile(shape, F32, tag=src.name, name="w_" + src.name)
            nc.sync.dma_start(out=t, in_=src[:])
            return t
        t_wxv, t_wzv = wtile([DM, DSL], i_wxv), wtile([DM, DSL], i_wzv)
        t_wxi, t_wzi = wtile([DM, DSL], i_wxi), wtile([DM, DSL], i_wzi)
        t_wsub = wtile([DM, DSL], i_wsub)
        t_w9 = wtile([DSL, 3, 9, DSL], i_w9)
        t_b72 = wtile([DSL, 3], i_b72)
        t_wpk = wtile([DSL, K, 14], i_wpk)
        t_wdtr = wtile([RNK, K, LANES], i_wdtr)
        t_dtb = wtile([LANES, K], i_dtb)
        t_asc = wtile([LANES, K], i_asc)
        t_rep24 = wtile([DSL, LANES], i_rep24)
        t_repb = wtile([NST, LANES], i_repb)
        t_m96 = wtile([LANES, DSL], i_m96)
        t_diagd = wtile([DSL, 2, DSL], i_diagd)
        t_f1 = wtile([DSL, 4, 12], i_f1)
        t_f2 = wtile([12, 2, 2, DM], i_f2)
        t_lnw = wtile([DM, 2, 4], i_lnw)
        t_wout = wtile([DM, 2, DM], i_wout)
        t_wz = wtile([DM, 4, DM], i_wz)
        t_onec = wtile([DM, 1], i_onec)
        t_oner = wtile([1, DM], i_oner)
        t_xvc = wtile([DM, PC], i_xvc)
        t_xic = wtile([DM, PC], i_xic)

        t_xvt = big.tile([DM, HW], F32)
        nc.sync.dma_start(out=t_xvt, in_=i_xvt[:])
        t_xit = big.tile([DM, HW], F32)
        nc.sync.dma_start(out=t_xit, in_=i_xit[:])

        # persistent SBUF
        t_xs = {m: big.tile([DSL, HW], F32, tag=f"xs_{m}", name=f"xs_{m}")
                for m in ("sub", "vi", "ir")}
        t_yvi = big.tile([DSL, HW], F32, tag="yvi")
        t_yir = big.tile([DSL, HW], F32, tag="yir")

        # =========== PHASE A: upstream (d-sharded) ===========
        with tc.tile_pool(name="pa1", bufs=1) as pa1, \
             tc.tile_pool(name="pa", bufs=3) as pa, \
             tc.tile_pool(name="pap", bufs=1, space="PSUM") as pap, \
             tc.tile_pool(name="pav", bufs=2, space="PSUM") as pav:
            t_xdiff = pa1.tile([DM, HW], F32, tag="xdiff")
            nc.vector.tensor_sub(t_xdiff[:], t_xvt[:], t_xit[:])

            pads = {}
            for mname in ("sub", "vi", "ir"):
                pads[mname] = pa1.tile([DSL, 50, 50], F32, tag=f"pad_{mname}",
                                        name=f"pad_{mname}")
                nc.vector.memset(pads[mname][:], 0.0)

            # z-branch (for chan-attn pooling only) + per-modality pooled stats
            t_zacc = pa1.tile([DSL, 2, len(ROWCHUNKS)], F32, tag="zacc")
            t_zc = {}
            for im, (mod, wz_, xt) in enumerate(
                    (("vi", t_wzv, t_xvt), ("ir", t_wzi, t_xit))):
                t_zc[mod] = pa1.tile([DSL, HW], F32, tag=f"zc{mod}", name=f"zc{mod}")
                for ic, (r0, nr) in enumerate(ROWCHUNKS):
                    cols = slice(r0 * W, (r0 + nr) * W)
                    p_z = pap.tile([DSL, RCH], F32, tag="pz")
                    nc.tensor.matmul(p_z[:, :nr * W], wz_[:], xt[:, cols],
                                     start=True, stop=True)
                    nc.scalar.activation(t_zc[mod][:, cols], p_z[:, :nr * W],
                                         AF.Silu, accum_out=t_zacc[:, im, ic:ic + 1])
            t_pool = pa1.tile([DSL, 4], F32, tag="tpool")  # (via,vim,ira,irm)
            nc.vector.tensor_reduce(t_pool[:, 0:1], t_zacc[:, 0, :],
                                    axis=mybir.AxisListType.X, op=OP.add)
            nc.vector.tensor_reduce(t_pool[:, 1:2], t_zc["vi"][:],
                                    axis=mybir.AxisListType.X, op=OP.max)
            nc.vector.tensor_reduce(t_pool[:, 2:3], t_zacc[:, 1, :],
                                    axis=mybir.AxisListType.X, op=OP.add)
            nc.vector.tensor_reduce(t_pool[:, 3:4], t_zc["ir"][:],
                                    axis=mybir.AxisListType.X, op=OP.max)
            # v1 partials [12, 4] -> zero-padded [14, 4]
            t_v1 = pa1.tile([14, 4], F32, tag="tv1")
            nc.vector.memset(t_v1[:], 0.0)
            p_v1 = pav.tile([12, 4], F32, tag="pv1")
            for j in range(4):
                nc.tensor.matmul(p_v1[:, j:j + 1], t_f1[:, j, :], t_pool[:, j:j + 1],
                                 start=True, stop=True)
            nc.scalar.copy(t_v1[0:12, :], p_v1[:])
            nc.sync.dma_start(out=d_r1i[:, 6 * HW:R1C], in_=t_v1[:])

            # x-branch in_proj -> padded conv input
            for g, (wg, xt) in enumerate(
                    (("sub", t_xdiff), ("vi", t_xvt), ("ir", t_xit))):
                wmat = {"sub": t_wsub, "vi": t_wxv, "ir": t_wxi}[wg]
                for (r0, nr) in ROWCHUNKS:
                    cols = slice(r0 * W, (r0 + nr) * W)
                    p_x = pap.tile([DSL, RCH], F32, tag="px")
                    nc.tensor.matmul(p_x[:, :nr * W], wmat[:], xt[:, cols],
                                     start=True, stop=True)
                    nc.scalar.copy(
                        pads[wg][:, 1 + r0:1 + r0 + nr, 1:49],
                        p_x[:, :nr * W].rearrange("p (a b) -> p a b", a=nr))

            # depthwise conv 3x3 (9 diag matmuls per group) + bias + silu
            for g, mod in enumerate(("sub", "vi", "ir")):
                for (r0, nr) in ROWCHUNKS:
                    p_c = pav.tile([DSL, RCH], F32, tag="pconv")
                    for tap in range(9):
                        dy, dx = tap // 3, tap % 3
                        nc.tensor.matmul(
                            p_c[:, :nr * W], t_w9[:, g, tap, :],
                            pads[mod][:, r0 + dy:r0 + dy + nr, dx:dx + 48],
                            start=(tap == 0), stop=(tap == 8))
                    nc.scalar.activation(
                        t_xs[mod][:, r0 * W:(r0 + nr) * W], p_c[:, :nr * W],
                        AF.Silu, bias=t_b72[:, g:g + 1], scale=1.0)

            # x_dbl partials -> r1 buffer (DMA straight from PSUM)
            for (k, half), coff in HALF_OFF.items():
                src = t_xs[("sub", "vi")[half] if k == 0 else
                           ("sub", "ir")[half] if k == 1 else
                           "vi" if k == 2 else "ir"]
                for (r0, nr) in ROWCHUNKS:
                    p_d = pap.tile([14, RCH], F32, tag="pxdbl")
                    nc.tensor.matmul(p_d[:, :nr * W], t_wpk[:, k, :],
                                     src[:, r0 * W:(r0 + nr) * W],
                                     start=True, stop=True)
                    t_xe = pa.tile([14, RCH], F32, tag="txdbl", name="t_xe")
                    nc.scalar.copy(t_xe[:, :nr * W], p_d[:, :nr * W])
                    nc.sync.dma_start(
                        out=d_r1i[:, coff + r0 * W:coff + (r0 + nr) * W],
                        in_=t_xe[:, :nr * W])

        # =========== R1: AllReduce ===========
        nc.gpsimd.collective_compute("AllReduce", OP.add, RG,
                                     ins=[d_r1i[:]], outs=[d_r1o[:]])

        # =========== PHASE B: scan middle ===========
        t_v1o = big.tile([12, 4], F32, tag="v1o")
        nc.sync.dma_start(out=t_v1o, in_=d_r1o[0:12, 6 * HW:6 * HW + 4])

        # tile col-spaces: t0=[k0sub|k0vi] r1-cols 0:4608, t1=[k1sub|k1ir]
        # 4608:9216, t2=[k2vi|k3ir] 9216:13824
        def xs_src(t, col):  # compact xs source for tile t at col (0..4607)
            half = col >= HW
            mod = (("sub", "vi"), ("sub", "ir"), ("vi", "ir"))[t][half]
            return t_xs[mod], col - HW if half else col

        with tc.tile_pool(name="pb", bufs=3) as pb, \
             tc.tile_pool(name="pbp", bufs=1, space="PSUM") as pbp, \
             tc.tile_pool(name="pby", bufs=2, space="PSUM") as pby:
            for t in range(3):
                r1off = t * L
                chunk_order = range(NCH) if t < 2 else range(NCH - 1, -1, -1)
                carry = None
                for c in chunk_order:
                    c0 = c * CH
                    # segment pieces within chunk: (start, end, k) in tile cols
                    k_lo = t if t < 2 else 2
                    k_hi = t if t < 2 else 3
                    if c0 >= HW:
                        pieces = [(c0, c0 + CH, k_hi)]
                    elif c0 + CH <= HW:
                        pieces = [(c0, c0 + CH, k_lo)]
                    else:
                        pieces = [(c0, HW, k_lo), (HW, c0 + CH, k_hi)]
                    rc = slice(r1off + c0, r1off + c0 + CH)

                    t_rR = pb.tile([RNK, CH], F32, tag="rR")
                    nc.sync.dma_start(out=t_rR, in_=d_r1o[0:RNK, rc])
                    t_rB = pb.tile([NST, CH], F32, tag="rB")
                    nc.sync.dma_start(out=t_rB, in_=d_r1o[RNK:RNK + NST, rc])
                    p_dts = pbp.tile([LANES, CH], F32, tag="dts")
                    for (s, e, k) in pieces:
                        nc.tensor.matmul(p_dts[:, s - c0:e - c0], t_wdtr[:, k, :],
                                         t_rR[:, s - c0:e - c0],
                                         start=True, stop=True)
                    t_et = pb.tile([LANES, CH], F32, tag="et")
                    for (s, e, k) in pieces:
                        nc.scalar.activation(t_et[:, s - c0:e - c0],
                                             p_dts[:, s - c0:e - c0], AF.Exp,
                                             bias=t_dtb[:, k:k + 1], scale=1.0)
                    t_delta = pb.tile([LANES, CH], F32, tag="delta")
                    nc.scalar.activation(t_delta[:], t_et[:], AF.Ln,
                                         bias=1.0, scale=1.0)
                    t_u = pb.tile([DSL, CH], F32, tag="u")
                    for (s, e, _k) in pieces:
                        src, sc = xs_src(t, s)
                        nc.vector.tensor_mul(t_u[:, s - c0:e - c0],
                                             t_delta[0:DSL, s - c0:e - c0],
                                             src[:, sc:sc + (e - s)])
                    p_u = pbp.tile([LANES, CH], F32, tag="urep")
                    nc.tensor.matmul(p_u[:], t_rep24[:], t_u[:], start=True, stop=True)
                    p_B = pbp.tile([LANES, CH], F32, tag="brep")
                    nc.tensor.matmul(p_B[:], t_repb[:], t_rB[:],
                                     start=True, stop=True)
                    t_bsb = pb.tile([LANES, CH], F32, tag="bsb")
                    nc.scalar.copy(t_bsb[:], p_B[:])
                    t_b = pb.tile([LANES, CH], F32, tag="b")
                    nc.vector.tensor_mul(t_b[:], p_u[:], t_bsb[:])
                    t_a = pb.tile([LANES, CH], F32, tag="a")
                    for (s, e, k) in pieces:
                        nc.scalar.activation(t_a[:, s - c0:e - c0],
                                             t_delta[:, s - c0:e - c0], AF.Exp,
                                             bias=0.0, scale=t_asc[:, k:k + 1])
                    t_h = pb.tile([LANES, CH], F32, tag="h")
                    if t < 2:
                        init = 0.0 if c == 0 else carry[:, CH - 1:CH]
                        nc.vector.tensor_tensor_scan(t_h[:], t_a[:], t_b[:], init,
                                                     OP.mult, OP.add)
                        carry = t_h
                    else:
                        # reverse scan; pieces processed right-to-left
                        for (s, e, k) in reversed(pieces):
                            sl = slice(s - c0, e - c0)
                            if e == L or e == HW:      # scan-time segment start
                                init = 0.0
                            else:
                                init = carry
                            nc.vector.tensor_tensor_scan(
                                t_h[:, sl][:, ::-1], t_a[:, sl][:, ::-1],
                                t_b[:, sl][:, ::-1], init, OP.mult, OP.add)
                            carry = t_h[:, s - c0:s - c0 + 1]

                    # y: only vi/ir halves feed the output
                    ypieces = [((s if t == 2 else max(s, HW)), e, k)
                               for (s, e, k) in pieces if t == 2 or e > HW]
                    if not ypieces:
                        continue
                    y0 = ypieces[0][0] - c0
                    y1 = ypieces[-1][1] - c0
                    t_rC = pb.tile([NST, CH], F32, tag="rC")
                    nc.sync.dma_start(out=t_rC[:, y0:y1],
                                      in_=d_r1o[RNK + NST:14, rc][:, y0:y1])
                    p_C = pbp.tile([LANES, CH], F32, tag="crep")
                    nc.tensor.matmul(p_C[:, y0:y1], t_repb[:],
                                     t_rC[:, y0:y1], start=True, stop=True)
                    t_hc = pb.tile([LANES, CH], F32, tag="hc")
                    nc.vector.tensor_mul(t_hc[:, y0:y1], t_h[:, y0:y1],
                                         p_C[:, y0:y1])
                    p_y = pby.tile([DSL, CH], F32, tag="y")
                    nc.tensor.matmul(p_y[:, y0:y1], t_m96[:], t_hc[:, y0:y1],
                                     start=True, stop=(t == 2))
                    if t < 2:   # D-skip, combined (D_k + D_{k+2}) on fwd tiles
                        for (s, e, _k) in ypieces:
                            src, sc = xs_src(t, s)
                            nc.tensor.matmul(p_y[:, s - c0:e - c0],
                                             t_diagd[:, t, :], src[:, sc:sc + e - s],
                                             start=False, stop=True)
                    # evacuate/accumulate into y_vi / y_ir
                    for (s, e, _k) in ypieces:
                        sl = slice(s - c0, e - c0)
                        if t == 0:
                            nc.scalar.copy(t_yvi[:, s - HW:e - HW], p_y[:, sl])
                        elif t == 1:
                            nc.scalar.copy(t_yir[:, s - HW:e - HW], p_y[:, sl])
                        elif s < HW:  # t2 k2 -> vi
                            nc.vector.tensor_add(t_yvi[:, s:e], t_yvi[:, s:e],
                                                 p_y[:, sl])
                        else:         # t2 k3 -> ir
                            nc.vector.tensor_add(t_yir[:, s - HW:e - HW],
                                                 t_yir[:, s - HW:e - HW], p_y[:, sl])

        # =========== A2A: reshard y channels -> positions ===========
        for j in range(NCORES):
            nc.sync.dma_start(out=d_a2i[j, 0:DSL, :],
                              in_=t_yvi[:, j * PC:(j + 1) * PC])
            nc.sync.dma_start(out=d_a2i[j, DSL:2 * DSL, :],
                              in_=t_yir[:, j * PC:(j + 1) * PC])
        nc.gpsimd.collective_compute("AllToAll", OP.bypass, RG,
                                     ins=[d_a2i[:]], outs=[d_a2o[:]])

        # =========== PHASE C: LN + gate + out (position-sharded) ===========
        with tc.tile_pool(name="pcq", bufs=2) as pcq, \
             tc.tile_pool(name="pcp", bufs=1, space="PSUM") as pcp:
            # gather y chunks [96, PC] x (2 chunks, 2 mods)
            t_y = {}
            for mod, roff in (("vi", 0), ("ir", DSL)):
                for ck in range(2):
                    ty = pcq.tile([DM, PC], F32, tag=f"y{mod}{ck}", name=f"y{mod}{ck}")
                    for jj in range(4):
                        j = ck * 4 + jj
                        nc.sync.dma_start(out=ty[jj * DSL:(jj + 1) * DSL, :],
                                          in_=d_a2o[j, roff:roff + DSL, :])
                    t_y[(mod, ck)] = ty
            # chan-attn scales s = 1 + sigmoid(f2 @ (relu(va)+relu(vm)))
            t_vr = pcq.tile([12, 4], F32, tag="vr")
            nc.scalar.activation(t_vr[:], t_v1o[:], AF.Relu)
            t_vw = pcq.tile([12, 2], F32, tag="vw")
            nc.vector.tensor_add(t_vw[:, 0:1], t_vr[:, 0:1], t_vr[:, 1:2])
            nc.vector.tensor_add(t_vw[:, 1:2], t_vr[:, 2:3], t_vr[:, 3:4])
            t_s = {}
            for ck in range(2):
                p_ca = pcp.tile([DM, 2], F32, tag="pca")
                for mod_i in range(2):
                    nc.tensor.matmul(p_ca[:, mod_i:mod_i + 1], t_f2[:, mod_i, ck, :],
                                     t_vw[:, mod_i:mod_i + 1], start=True, stop=True)
                t_e = pcq.tile([DM, 2], F32, tag="cae")
                nc.scalar.activation(t_e[:], p_ca[:], AF.Exp, bias=0.0, scale=-1.0)
                nc.vector.tensor_scalar_add(t_e[:], t_e[:], 1.0)
                t_r = pcq.tile([DM, 2], F32, tag=f"car{ck}", name=f"car{ck}")
                nc.vector.reciprocal(t_r[:], t_e[:])          # sigmoid
                nc.vector.tensor_scalar_add(t_r[:], t_r[:], 1.0)  # 1 + sigmoid
                t_s[ck] = t_r
            # z recompute at my positions: z = x @ Wz, silu via exp+recip
            t_z = {}
            for zi, (mod, ck) in enumerate(
                    (("vi", 0), ("vi", 1), ("ir", 0), ("ir", 1))):
                xt = t_xvc if mod == "vi" else t_xic
                p_z = pcp.tile([DM, PC], F32, tag="pz2")
                nc.tensor.matmul(p_z[:], t_wz[:, zi, :], xt[:],
                                 start=True, stop=True)
                t_e = pcq.tile([DM, PC], F32, tag="ze")
                nc.scalar.activation(t_e[:], p_z[:], AF.Exp, bias=0.0, scale=-1.0)
                nc.vector.tensor_scalar_add(t_e[:], t_e[:], 1.0)
                t_r = pcq.tile([DM, PC], F32, tag="zr")
                nc.vector.reciprocal(t_r[:], t_e[:])
                tz = pcq.tile([DM, PC], F32, tag=f"z{zi}", name=f"z{zi}")
                nc.vector.tensor_mul(tz[:], p_z[:], t_r[:])
                t_z[(mod, ck)] = tz
            # LN per modality
            t_fin = {}
            for mod in ("vi", "ir"):
                p_s1 = pcp.tile([1, PC], F32, tag="s1")
                p_s2 = pcp.tile([1, PC], F32, tag="s2")
                for ck in range(2):
                    nc.tensor.matmul(p_s1[:], t_onec[:],
                                     t_y[(mod, ck)][:], start=(ck == 0),
                                     stop=(ck == 1))
                for ck in range(2):
                    t_sq = pcq.tile([DM, PC], F32, tag="sq")
                    nc.scalar.activation(t_sq[:], t_y[(mod, ck)][:], AF.Square)
                    nc.tensor.matmul(p_s2[:], t_onec[:],
                                     t_sq[:], start=(ck == 0), stop=(ck == 1))
                t_mu = pcq.tile([1, PC], F32, tag="mu")
                nc.vector.tensor_scalar_mul(t_mu[:], p_s1[:], 1.0 / DI)
                t_musq = pcq.tile([1, PC], F32, tag="musq")
                nc.vector.tensor_mul(t_musq[:], t_mu[:], t_mu[:])
                t_var = pcq.tile([1, PC], F32, tag="var")
                nc.vector.scalar_tensor_tensor(t_var[:], p_s2[:], 1.0 / DI,
                                               t_musq[:], OP.mult, OP.subtract)
                t_eps = pcq.tile([1, 1], F32, tag="eps")
                nc.vector.memset(t_eps[:], 1e-5)
                t_lnv = pcq.tile([1, PC], F32, tag="lnv")
                nc.scalar.activation(t_lnv[:], t_var[:], AF.Ln, bias=t_eps[:], scale=1.0)
                t_rstd = pcq.tile([1, PC], F32, tag="rstd")
                nc.scalar.activation(t_rstd[:], t_lnv[:], AF.Exp, bias=0.0, scale=-0.5)
                t_mur = pcq.tile([1, PC], F32, tag="mur")
                nc.vector.tensor_mul(t_mur[:], t_mu[:], t_rstd[:])
                p_q = pcp.tile([DM, PC], F32, tag="pq")
                nc.tensor.matmul(p_q[:], t_oner[:], t_rstd[:], start=True, stop=True)
                p_m = pcp.tile([DM, PC], F32, tag="pm")
                nc.tensor.matmul(p_m[:], t_oner[:], t_mur[:], start=True, stop=True)
                gb = {"vi": (0, 1), "ir": (2, 3)}[mod]
                for ck in range(2):
                    t_t = pcq.tile([DM, PC], F32, tag="lt")
                    nc.vector.tensor_mul(t_t[:], t_y[(mod, ck)][:], p_q[:])
                    t_t2 = pcq.tile([DM, PC], F32, tag="lt2")
                    nc.vector.tensor_sub(t_t2[:], t_t[:], p_m[:])
                    t_yn = pcq.tile([DM, PC], F32, tag="yn")
                    nc.scalar.activation(t_yn[:], t_t2[:], AF.Identity,
                                         bias=t_lnw[:, ck, gb[1]:gb[1] + 1],
                                         scale=t_lnw[:, ck, gb[0]:gb[0] + 1])
                    # gate: fin += yn * z * s
                    t_m1 = pcq.tile([DM, PC], F32, tag="m1")
                    nc.vector.tensor_mul(t_m1[:], t_yn[:], t_z[(mod, ck)][:])
                    if mod == "vi":
                        t_f = pcq.tile([DM, PC], F32, tag=f"fin{ck}", name=f"fin{ck}")
                        nc.vector.tensor_scalar_mul(t_f[:], t_m1[:],
                                                    t_s[ck][:, 0:1])
                        t_fin[ck] = t_f
                    else:
                        nc.vector.scalar_tensor_tensor(t_fin[ck][:], t_m1[:],
                                                       t_s[ck][:, 1:2], t_fin[ck][:],
                                                       OP.mult, OP.add)
            p_o = pcp.tile([DM, PC], F32, tag="po")
            for ck in range(2):
                nc.tensor.matmul(p_o[:], t_wout[:, ck, :], t_fin[ck][:],
                                 start=(ck == 0), stop=(ck == 1))
            t_o = pcq.tile([DM, PC], F32, tag="o")
            nc.scalar.copy(t_o[:], p_o[:])
            nc.sync.dma_start(out=o_out[:], in_=t_o[:])

    nc.finalize()
    return nc


def _prep_inputs(inputs):
    """Host-side prep: slice/transpose weights per core. Returns in_maps."""
    g = {k: np.asarray(v, dtype=np.float32) for k, v in inputs.items()}
    x_vi = g["x_vi"].reshape(HW, DM)
    x_ir = g["x_ir"].reshape(HW, DM)
    xvt = np.ascontiguousarray(x_vi.T)
    xit = np.ascontiguousarray(x_ir.T)
    A = -np.exp(g["A_logs"]).reshape(K, DI, NST)
    Ds = g["Ds"].reshape(K, DI)
    in_maps = []
    for c in range(NCORES):
        S = slice(c * DSL, (c + 1) * DSL)
        m = {}
        m["xvt"], m["xit"] = xvt, xit
        m["wxv"] = np.ascontiguousarray(g["W_vi"][S].T)
        m["wzv"] = np.ascontiguousarray(g["W_vi"][DI:][S].T)
        m["wxi"] = np.ascontiguousarray(g["W_ir"][S].T)
        m["wzi"] = np.ascontiguousarray(g["W_ir"][DI:][S].T)
        m["wsub"] = np.ascontiguousarray(g["W_sub"][S].T)
        w9 = np.zeros((DSL, 3, 9, DSL), np.float32)
        b72 = np.zeros((DSL, 3), np.float32)
        for gi, nm in enumerate(("sub", "vi", "ir")):
            cw = g[f"conv_w_{nm}"][S, 0]      # [DSL, 3, 3]
            for tap in range(9):
                for d in range(DSL):
                    w9[d, gi, tap, d] = cw[d, tap // 3, tap % 3]
            b72[:, gi] = g[f"conv_b_{nm}"][S]
        m["w9"], m["b72"] = w9, b72
        m["wpk"] = np.ascontiguousarray(
            g["x_proj_weight"][:, :, S].transpose(2, 0, 1))  # [DSL, K, 14]
        wdtr = np.zeros((RNK, K, LANES), np.float32)
        dtb = np.zeros((LANES, K), np.float32)
        asc = np.zeros((LANES, K), np.float32)
        for k in range(K):
            for n in range(NST):
                for d in range(DSL):
                    lane = n * DSL + d
                    wdtr[:, k, lane] = g["dt_projs_weight"][k, c * DSL + d, :]
                    dtb[lane, k] = g["dt_projs_bias"][k, c * DSL + d]
                    asc[lane, k] = A[k, c * DSL + d, n]
        m["wdtr"], m["dtb"], m["asc"] = wdtr, dtb, asc
        rep24 = np.zeros((DSL, LANES), np.float32)
        repb = np.zeros((NST, LANES), np.float32)
        m96 = np.zeros((LANES, DSL), np.float32)
        for n in range(NST):
            for d in range(DSL):
                rep24[d, n * DSL + d] = 1
                repb[n, n * DSL + d] = 1
                m96[n * DSL + d, d] = 1
        m["rep24"], m["repb"], m["m96"] = rep24, repb, m96
        diagd = np.zeros((DSL, 2, DSL), np.float32)
        np.fill_diagonal(diagd[:, 0, :], Ds[0, S] + Ds[2, S])
        np.fill_diagonal(diagd[:, 1, :], Ds[1, S] + Ds[3, S])
        m["diagd"] = diagd
        f1 = np.zeros((DSL, 4, 12), np.float32)
        f1[:, 0] = g["ca_vi_f1"][:, S].T / HW
        f1[:, 1] = g["ca_vi_f1"][:, S].T
        f1[:, 2] = g["ca_ir_f1"][:, S].T / HW
        f1[:, 3] = g["ca_ir_f1"][:, S].T
        m["f1"] = f1
        f2 = np.zeros((12, 2, 2, DM), np.float32)
        for ck in range(2):
            f2[:, 0, ck] = g["ca_vi_f2"][ck * DM:(ck + 1) * DM].T
            f2[:, 1, ck] = g["ca_ir_f2"][ck * DM:(ck + 1) * DM].T
        m["f2"] = f2
        lnw = np.zeros((DM, 2, 4), np.float32)
        for ck in range(2):
            cs = slice(ck * DM, (ck + 1) * DM)
            lnw[:, ck, 0] = g["ln_vi_g"][cs]
            lnw[:, ck, 1] = g["ln_vi_b"][cs]
            lnw[:, ck, 2] = g["ln_ir_g"][cs]
            lnw[:, ck, 3] = g["ln_ir_b"][cs]
        m["lnw"] = lnw
        wout = np.zeros((DM, 2, DM), np.float32)
        for ck in range(2):
            wout[:, ck] = g["W_out"][:, ck * DM:(ck + 1) * DM].T
        m["wout"] = wout
        wz = np.zeros((DM, 4, DM), np.float32)
        wz[:, 0] = g["W_vi"][DI:][0:DM].T
        wz[:, 1] = g["W_vi"][DI:][DM:DI].T
        wz[:, 2] = g["W_ir"][DI:][0:DM].T
        wz[:, 3] = g["W_ir"][DI:][DM:DI].T
        m["wz"] = wz
        m["onec"] = np.ones((DM, 1), np.float32)
        m["oner"] = np.ones((1, DM), np.float32)
        m["xvc"] = np.ascontiguousarray(xvt[:, c * PC:(c + 1) * PC])
        m["xic"] = np.ascontiguousarray(xit[:, c * PC:(c + 1) * PC])
        in_maps.append(m)
    return in_maps


def kernel(**inputs):
    if "nc" not in _cache:
        _cache["nc"] = _build()
    nc = _cache["nc"]
    in_maps = _prep_inputs(inputs)
    res = run_bass_kernel_spmd(nc, in_maps, core_ids=list(range(NCORES)))
    out = np.zeros((DM, HW), np.float32)
    for c in range(NCORES):
        out[:, c * PC:(c + 1) * PC] = res.results[c]["out"]
    return out.T.reshape(B, H, W, DM).astype(np.float32)



# revision 14
# speedup vs baseline: 1.8960x; 1.8960x over previous
"""Trainium2 Bass kernel for the DSSM (dual-modality Mamba-style 2D selective
scan) module. 8-core SPMD: scan channels d-sharded (24/core x 4 directions),
upstream in_proj/dwconv d-sharded, downstream LN/out position-sharded.
Cross-core: 3 chunked AllReduces (x_dbl partials, one per scan tile,
overlapped with compute) + tiny v1 AllReduce + one AllToAll (y reshard).
"""
import sys
sys.path.insert(0, "/opt/trn_rl_repo")
import numpy as np
import concourse.bass as bass
from concourse import mybir
from concourse.bacc import Bacc
from concourse.tile import TileContext
from concourse.bass_utils import run_bass_kernel_spmd

F32 = mybir.dt.float32
F32R = mybir.dt.float32r
AF = mybir.ActivationFunctionType
OP = mybir.AluOpType

NCORES = 8
RG = [list(range(NCORES))]
B, H, W = 1, 48, 48
HW = H * W                      # 2304
L = 2 * HW                      # 4608
DM = 96                         # d_model
DI = 192                        # d_inner
NST = 4                         # d_state
RNK = 6                         # dt_rank
K = 4
DSL = DI // NCORES              # 24 channels per core
LANES = NST * DSL               # 96 scan lanes (lane = n*DSL + d)
CH = 512                        # phase-B PSUM column chunk
NCH = L // CH                   # 9
PC = HW // NCORES               # 288 positions per core (phase C)
RCH = 480                       # phase-A chunk = 10 image rows
ROWCHUNKS = [(0, 10), (10, 10), (20, 10), (30, 10), (40, 8)]
XOFF = {"sub": 0, "vi": 32, "ir": 64}   # row block in stacked xs96
# tile t: (half0 mod, half1 mod); k per segment = t (t<2) else 2/3
TMODS = (("sub", "vi"), ("sub", "ir"), ("vi", "ir"))

_cache = {}


def _build():
    nc = Bacc(trn_type="TRN2", num_devices=NCORES)
    EIn = dict(kind="ExternalInput")
    i_xvt = nc.dram_tensor("xvt", [DM, HW], F32, **EIn)
    i_xit = nc.dram_tensor("xit", [DM, HW], F32, **EIn)
    i_w48v = nc.dram_tensor("w48v", [DM, 64], F32, **EIn)  # [x@0|z@32] lhsT
    i_w48i = nc.dram_tensor("w48i", [DM, 64], F32, **EIn)  # [x@0|z@32]
    i_wsub = nc.dram_tensor("wsub", [DM, DSL], F32, **EIn)
    i_w72 = nc.dram_tensor("w72", [96, 9, 96], F32, **EIn)   # conv block-diag
    i_b72 = nc.dram_tensor("b72", [96, 1], F32, **EIn)       # conv bias stacked
    i_w84 = nc.dram_tensor("w84", [96, 3, 28], F32, **EIn)   # x_dbl per tile
    i_wdtr = nc.dram_tensor("wdtr", [RNK, K, LANES], F32, **EIn)
    i_dtb = nc.dram_tensor("dtb", [LANES, K], F32, **EIn)
    i_asc = nc.dram_tensor("asc", [LANES, K], F32, **EIn)
    i_m96 = nc.dram_tensor("m96", [LANES, DSL], F32, **EIn)
    i_diagd = nc.dram_tensor("diagd", [96, 2, DSL], F32, **EIn)  # (vi,ir) D
    i_f1 = nc.dram_tensor("f1", [DSL, 4, 12], F32, **EIn)   # (via,vim,ira,irm)
    i_f2 = nc.dram_tensor("f2", [12, 2, 2, DM], F32, **EIn)  # (mod, chunk, out)
    i_lnw = nc.dram_tensor("lnw", [DM, 2, 4], F32, **EIn)    # per chunk g/b
    i_wout = nc.dram_tensor("wout", [DM, 2, DM], F32, **EIn)
    i_wz = nc.dram_tensor("wz", [DM, 4, DM], F32, **EIn)     # z lhsT
    i_onec = nc.dram_tensor("onec", [DM, 1], F32, **EIn)
    i_oner = nc.dram_tensor("oner", [1, DM], F32, **EIn)
    i_xvc = nc.dram_tensor("xvc", [DM, PC], F32, **EIn)
    i_xic = nc.dram_tensor("xic", [DM, PC], F32, **EIn)
    o_out = nc.dram_tensor("out", [DM, PC], F32, kind="ExternalOutput")
    # collective DRAM buffers: per scan tile [half, row, col]
    d_ri = [nc.dram_tensor(f"d_ri{t}", [2, 14, HW], F32) for t in range(3)]
    d_ro = [nc.dram_tensor(f"d_ro{t}", [2, 14, HW], F32, addr_space="Shared")
            for t in range(3)]
    d_xsr = {m_: nc.dram_tensor(f"d_xsr_{m_}", [LANES, HW], F32)
             for m_ in ("sub", "vi", "ir")}
    d_v1i = nc.dram_tensor("d_v1i", [12, 4], F32)
    d_v1o = nc.dram_tensor("d_v1o", [12, 4], F32, addr_space="Shared")
    d_a2i = nc.dram_tensor("d_a2i", [NCORES, 2 * DSL, PC], F32)
    d_a2o = nc.dram_tensor("d_a2o", [NCORES, 2 * DSL, PC], F32)

    def mmr(out, lhsT, rhs, **kw):
        # float32r matmul: 1 cycle/row (vs 4 for fp32) when free dim >= 256
        nc.tensor.matmul(out, lhsT.bitcast(F32R), rhs.bitcast(F32R), **kw)

    import contextlib
    with TileContext(nc) as tc, contextlib.ExitStack() as ctx:
        wpool = ctx.enter_context(tc.tile_pool(name="weights", bufs=1))
        big = ctx.enter_context(tc.tile_pool(name="big", bufs=1))

        def wtile(shape, src, rnd=False):
            t = wpool.tile(shape, F32, tag=src.name, name="w_" + src.name)
            if rnd:
                nc.sync.dma_start(out=t[:].bitcast(F32R),
                                  in_=src[:].bitcast(F32R))
            else:
                nc.sync.dma_start(out=t, in_=src[:])
            return t
        t_w48v = wtile([DM, 64], i_w48v, True)
        t_w48i = wtile([DM, 64], i_w48i, True)
        t_wsub = wtile([DM, DSL], i_wsub, True)
        t_w72 = wtile([96, 9, 96], i_w72, True)
        t_b72 = wtile([96, 1], i_b72)
        t_w84 = wtile([96, 3, 28], i_w84, True)
        t_wdtr = wtile([RNK, K, LANES], i_wdtr, True)
        t_dtb = wtile([LANES, K], i_dtb)
        t_asc = wtile([LANES, K], i_asc)
        t_m96 = wtile([LANES, DSL], i_m96, True)
        t_diagd = wtile([96, 2, DSL], i_diagd, True)
        t_f1 = wtile([DSL, 4, 12], i_f1)
        t_f2 = wtile([12, 2, 2, DM], i_f2)
        t_lnw = wtile([DM, 2, 4], i_lnw)
        t_wout = wtile([DM, 2, DM], i_wout, True)
        t_wz = wtile([DM, 4, DM], i_wz, True)
        t_onec = wtile([DM, 1], i_onec)
        t_oner = wtile([1, DM], i_oner)
        t_xvc = wtile([DM, PC], i_xvc, True)
        t_xic = wtile([DM, PC], i_xic, True)

        # persistent SBUF
        t_xs72 = big.tile([96, HW], F32, tag="xs72")   # (sub|vi|ir) @ 0/32/64
        t_yvi = big.tile([DSL, HW], F32, tag="yvi")
        t_yir = big.tile([DSL, HW], F32, tag="yir")

        # =========== PHASE A: upstream (d-sharded) ===========
        with tc.tile_pool(name="pa1", bufs=1) as pa1, \
             tc.tile_pool(name="pa", bufs=3) as pa, \
             tc.tile_pool(name="pap", bufs=2, space="PSUM") as pap, \
             tc.tile_pool(name="pas", bufs=2, space="PSUM") as pas, \
             tc.tile_pool(name="pav", bufs=2, space="PSUM") as pav, \
             tc.tile_pool(name="pav1", bufs=1, space="PSUM") as pav1:
            t_xvt = pa1.tile([DM, HW], F32, tag="xvt")
            nc.sync.dma_start(out=t_xvt[:].bitcast(F32R),
                              in_=i_xvt[:].bitcast(F32R))
            t_xit = pa1.tile([DM, HW], F32, tag="xit")
            nc.sync.dma_start(out=t_xit[:].bitcast(F32R),
                              in_=i_xit[:].bitcast(F32R))
            t_xdiff = pa1.tile([DM, HW], F32, tag="xdiff")
            nc.vector.tensor_sub(t_xdiff[:].bitcast(F32R), t_xvt[:], t_xit[:])

            pad72 = pa1.tile([96, 50, 50], F32, tag="pad72")
            nc.vector.memset(pad72[:], 0.0)

            # in_proj: merged [zv|xv] / [zi|xi] matmuls + sub
            t_zacc = pa1.tile([DSL, 2, len(ROWCHUNKS)], F32, tag="zacc")
            t_zc = {"vi": pa1.tile([DSL, HW], F32, tag="zcvi", name="zcvi"),
                    "ir": pa1.tile([DSL, HW], F32, tag="zcir", name="zcir")}
            for im, (mod, w48, xt) in enumerate(
                    (("vi", t_w48v, t_xvt), ("ir", t_w48i, t_xit))):
                for ic, (r0, nr) in enumerate(ROWCHUNKS):
                    cols = slice(r0 * W, (r0 + nr) * W)
                    p48 = pap.tile([64, RCH], F32, tag="p48")
                    mmr(p48[:, :nr * W], w48[:], xt[:, cols],
                        start=True, stop=True)
                    nc.scalar.activation(t_zc[mod][:, cols],
                                         p48[32:56, :nr * W], AF.Silu,
                                         accum_out=t_zacc[:, im, ic:ic + 1])
                    o = XOFF[mod]
                    nc.scalar.copy(
                        pad72[o:o + DSL, 1 + r0:1 + r0 + nr, 1:49]
                        .bitcast(F32R),
                        p48[0:DSL, :nr * W]
                        .rearrange("p (a b) -> p a b", a=nr))
            for (r0, nr) in ROWCHUNKS:
                cols = slice(r0 * W, (r0 + nr) * W)
                p24 = pas.tile([28, RCH], F32, tag="px")
                mmr(p24[0:DSL, :nr * W], t_wsub[:], t_xdiff[:, cols],
                    start=True, stop=True)
                nc.scalar.copy(
                    pad72[0:DSL, 1 + r0:1 + r0 + nr, 1:49].bitcast(F32R),
                    p24[0:DSL, :nr * W].rearrange("p (a b) -> p a b", a=nr))

            # chan-attn pooled stats -> v1 partials -> tiny AR
            t_pool = pa1.tile([DSL, 4], F32, tag="tpool")
            nc.vector.tensor_reduce(t_pool[:, 0:1], t_zacc[:, 0, :],
                                    axis=mybir.AxisListType.X, op=OP.add)
            nc.vector.tensor_reduce(t_pool[:, 1:2], t_zc["vi"][:],
                                    axis=mybir.AxisListType.X, op=OP.max)
            nc.vector.tensor_reduce(t_pool[:, 2:3], t_zacc[:, 1, :],
                                    axis=mybir.AxisListType.X, op=OP.add)
            nc.vector.tensor_reduce(t_pool[:, 3:4], t_zc["ir"][:],
                                    axis=mybir.AxisListType.X, op=OP.max)
            t_v1 = pa1.tile([12, 4], F32, tag="tv1")
            p_v1 = pav1.tile([12, 4], F32, tag="pv1")
            for j in range(4):
                nc.tensor.matmul(p_v1[:, j:j + 1], t_f1[:, j, :],
                                 t_pool[:, j:j + 1], start=True, stop=True)
            nc.scalar.copy(t_v1[:], p_v1[:])
            nc.sync.dma_start(out=d_v1i[:], in_=t_v1[:])

            # depthwise conv 3x3: block-diag 72-channel, 9 taps
            for (r0, nr) in ROWCHUNKS:
                p_c = pav.tile([96, RCH], F32, tag="pconv")
                for tap in range(9):
                    dy, dx = tap // 3, tap % 3
                    mmr(p_c[:, :nr * W], t_w72[:, tap, :],
                        pad72[:, r0 + dy:r0 + dy + nr, dx:dx + 48],
                        start=(tap == 0), stop=(tap == 8))
                nc.scalar.activation(
                    t_xs72[:, r0 * W:(r0 + nr) * W].bitcast(F32R),
                    p_c[:, :nr * W], AF.Silu, bias=t_b72[:, 0:1], scale=1.0)
            # xs lane-replicas to DRAM for phase-B u broadcast loads
            for m_ in ("sub", "vi", "ir"):
                o = XOFF[m_]
                for n in range(NST):
                    nc.sync.dma_start(out=d_xsr[m_][n * DSL:(n + 1) * DSL, :],
                                      in_=t_xs72[o:o + DSL, :])

            # x_dbl partials per scan tile -> DRAM -> chunked AllReduce
            for tg in range(3):
                for (r0, nr) in ROWCHUNKS:
                    cols = slice(r0 * W, (r0 + nr) * W)
                    p84 = pas.tile([28, RCH], F32, tag="px")
                    mmr(p84[:, :nr * W], t_w84[:, tg, :], t_xs72[:, cols],
                        start=True, stop=True)
                    t_xe = pa.tile([28, RCH], F32, tag="txdbl", name="t_xe")
                    nc.scalar.copy(t_xe[:, :nr * W], p84[:, :nr * W])
                    nc.sync.dma_start(
                        out=d_ri[tg][:, :, r0 * W:(r0 + nr) * W],
                        in_=t_xe[:, :nr * W]
                        .rearrange("(h p) c -> h p c", h=2))
                nc.gpsimd.collective_compute(
                    "AllReduce", OP.add, RG,
                    ins=[d_ri[tg][:]], outs=[d_ro[tg][:]])
            nc.gpsimd.collective_compute("AllReduce", OP.add, RG,
                                         ins=[d_v1i[:]], outs=[d_v1o[:]])

        # =========== PHASE B: scan middle (full-tile staging) ===========
        with tc.tile_pool(name="pb", bufs=1) as pb, \
             tc.tile_pool(name="pb2", bufs=2) as pb2, \
             tc.tile_pool(name="pbp", bufs=2, space="PSUM") as pbp, \
             tc.tile_pool(name="pby", bufs=2, space="PSUM") as pby:
            for t in range(3):
                segs = ([(0, L, t)] if t < 2 else
                        [(0, HW, 2), (HW, L, 3)])  # (start, end, k) tile cols
                yc0 = HW if t < 2 else 0            # y column span
                dro = d_ro[t]
                # ---- full-tile staged loads (DMA) ----
                t_rR = pb2.tile([RNK, L], F32, tag="rR")
                nc.sync.dma_start(
                    out=t_rR[:].rearrange("p (h c) -> p h c", h=2)
                    .bitcast(F32R),
                    in_=dro[:, 0:RNK, :].transpose([1, 0, 2]).bitcast(F32R))
                # B replicated to lanes (lane = n*DSL + d) via broadcast DMA
                t_Brep = pb.tile([LANES, L], F32, tag="Brep")
                for half in (0, 1):
                    nc.sync.dma_start(
                        out=t_Brep[:, half * HW:(half + 1) * HW],
                        in_=dro[half, RNK:RNK + NST, :]
                        .unsqueeze(1).broadcast_to([NST, DSL, HW]))
                # xs replicated to lanes (4 plain copies per half)
                t_urep = pb.tile([LANES, L], F32, tag="urep")
                for half in (0, 1):
                    nc.sync.dma_start(
                        out=t_urep[:, half * HW:(half + 1) * HW],
                        in_=d_xsr[TMODS[t][half]][:])
                # C replicated (y cols only)
                t_Crep = pb.tile([LANES, L], F32, tag="Crep")
                for half in ((1,) if t < 2 else (0, 1)):
                    nc.sync.dma_start(
                        out=t_Crep[:, half * HW:(half + 1) * HW],
                        in_=dro[half, RNK + NST:14, :]
                        .unsqueeze(1).broadcast_to([NST, DSL, HW]))

                # ---- dts chunks: matmul + Exp evac (exp table) ----
                t_et = pb2.tile([LANES, L], F32, tag="eta")
                for c in range(NCH):
                    c0 = c * CH
                    pieces = [(max(s, c0), min(e, c0 + CH), k)
                              for (s, e, k) in segs
                              if e > c0 and s < c0 + CH]
                    p_dts = pbp.tile([LANES, CH], F32, tag="dts")
                    for (s, e, k) in pieces:
                        mmr(p_dts[:, s - c0:e - c0], t_wdtr[:, k, :],
                            t_rR[:, s:e], start=True, stop=True)
                    for (s, e, k) in pieces:
                        nc.scalar.activation(t_et[:, s:e],
                                             p_dts[:, s - c0:e - c0], AF.Exp,
                                             bias=t_dtb[:, k:k + 1], scale=1.0)
                # ---- delta = softplus (ln table), then a = exp(asc*delta) ----
                t_delta = pb.tile([LANES, L], F32, tag="delta")
                nc.scalar.activation(t_delta[:], t_et[:], AF.Ln,
                                     bias=1.0, scale=1.0)
                t_a = pb2.tile([LANES, L], F32, tag="eta")  # reuse et buffer
                for (s, e, k) in segs:
                    nc.scalar.activation(t_a[:, s:e], t_delta[:, s:e], AF.Exp,
                                         bias=0.0, scale=t_asc[:, k:k + 1])
                # ---- b = delta * B_rep * xs_rep (in-place, DVE + gpsimd) ----
                SPL = 3072   # DVE gets 2/3, gpsimd 1/3 (eff ~0.42)
                nc.vector.tensor_mul(t_Brep[:, 0:SPL], t_delta[:, 0:SPL],
                                     t_Brep[:, 0:SPL])
                nc.gpsimd.tensor_mul(t_Brep[:, SPL:L], t_delta[:, SPL:L],
                                     t_Brep[:, SPL:L])
                nc.vector.tensor_mul(t_Brep[:, 0:SPL], t_Brep[:, 0:SPL],
                                     t_urep[:, 0:SPL])
                nc.gpsimd.tensor_mul(t_Brep[:, SPL:L], t_Brep[:, SPL:L],
                                     t_urep[:, SPL:L])
                # ---- scan ----
                t_h = pb.tile([LANES, L], F32, tag="h")
                if t < 2:
                    nc.vector.tensor_tensor_scan(t_h[:].bitcast(F32R), t_a[:],
                                                 t_Brep[:], 0.0,
                                                 OP.mult, OP.add)
                else:
                    for (s, e, k) in segs:   # reversed scans, fresh state
                        nc.vector.tensor_tensor_scan(
                            t_h[:, s:e][:, ::-1].bitcast(F32R),
                            t_a[:, s:e][:, ::-1],
                            t_Brep[:, s:e][:, ::-1], 0.0, OP.mult, OP.add)
                # ---- hc = h * C_rep (in-place into h), y cols only ----
                nc.gpsimd.tensor_mul(t_h[:, yc0:L].bitcast(F32R),
                                     t_h[:, yc0:L], t_Crep[:, yc0:L])
                # ---- y = m96 @ hc (+ D skip) -> accumulate into yvi/yir ----
                c0 = yc0
                while c0 < L:
                    c1 = min(c0 + CH, L)
                    p_y = pby.tile([DSL, CH], F32, tag="y")
                    mmr(p_y[:, :c1 - c0], t_m96[:], t_h[:, c0:c1],
                        start=True, stop=(t == 2))
                    if t < 2:   # D-skip, combined (D_k + D_{k+2}) on fwd tiles
                        o = XOFF[TMODS[t][1]]
                        mmr(p_y[:, :c1 - c0], t_diagd[o:o + DSL, t, :],
                            t_xs72[o:o + DSL, c0 - HW:c1 - HW],
                            start=False, stop=True)
                        dst = t_yvi if t == 0 else t_yir
                        nc.scalar.copy(dst[:, c0 - HW:c1 - HW],
                                       p_y[:, :c1 - c0])
                    else:
                        for (s, e) in ((c0, min(c1, HW)), (max(c0, HW), c1)):
                            if e <= s:
                                continue
                            if e <= HW:
                                nc.vector.tensor_add(
                                    t_yvi[:, s:e], t_yvi[:, s:e],
                                    p_y[:, s - c0:e - c0])
                            else:
                                nc.vector.tensor_add(
                                    t_yir[:, s - HW:e - HW],
                                    t_yir[:, s - HW:e - HW],
                                    p_y[:, s - c0:e - c0])
                    c0 = c1

        # =========== A2A: reshard y channels -> positions ===========
        for j in range(NCORES):
            nc.sync.dma_start(out=d_a2i[j, 0:DSL, :],
                              in_=t_yvi[:, j * PC:(j + 1) * PC])
            nc.sync.dma_start(out=d_a2i[j, DSL:2 * DSL, :],
                              in_=t_yir[:, j * PC:(j + 1) * PC])
        nc.gpsimd.collective_compute("AllToAll", OP.bypass, RG,
                                     ins=[d_a2i[:]], outs=[d_a2o[:]])
        t_v1o = big.tile([12, 4], F32, tag="v1o")
        nc.sync.dma_start(out=t_v1o, in_=d_v1o[:])

        # =========== PHASE C: LN + gate + out (position-sharded) ===========
        with tc.tile_pool(name="pcq", bufs=2) as pcq, \
             tc.tile_pool(name="pcp", bufs=1, space="PSUM") as pcp:
            # gather y chunks [96, PC] x (2 chunks, 2 mods)
            t_y = {}
            for mod, roff in (("vi", 0), ("ir", DSL)):
                for ck in range(2):
                    ty = pcq.tile([DM, PC], F32, tag=f"y{mod}{ck}",
                                  name=f"y{mod}{ck}")
                    for jj in range(4):
                        j = ck * 4 + jj
                        nc.sync.dma_start(
                            out=ty[jj * DSL:(jj + 1) * DSL, :].bitcast(F32R),
                            in_=d_a2o[j, roff:roff + DSL, :].bitcast(F32R))
                    t_y[(mod, ck)] = ty
            # chan-attn scales s = 1 + sigmoid(f2 @ (relu(va)+relu(vm)))
            t_vr = pcq.tile([12, 4], F32, tag="vr")
            nc.scalar.activation(t_vr[:], t_v1o[:], AF.Relu)
            t_vw = pcq.tile([12, 2], F32, tag="vw")
            nc.vector.tensor_add(t_vw[:, 0:1], t_vr[:, 0:1], t_vr[:, 1:2])
            nc.vector.tensor_add(t_vw[:, 1:2], t_vr[:, 2:3], t_vr[:, 3:4])
            t_s = {}
            for ck in range(2):
                p_ca = pcp.tile([DM, 2], F32, tag="pca")
                for mod_i in range(2):
                    nc.tensor.matmul(p_ca[:, mod_i:mod_i + 1],
                                     t_f2[:, mod_i, ck, :],
                                     t_vw[:, mod_i:mod_i + 1],
                                     start=True, stop=True)
                t_e = pcq.tile([DM, 2], F32, tag="cae")
                nc.scalar.activation(t_e[:], p_ca[:], AF.Exp,
                                     bias=0.0, scale=-1.0)
                nc.vector.tensor_scalar_add(t_e[:], t_e[:], 1.0)
                t_r = pcq.tile([DM, 2], F32, tag=f"car{ck}", name=f"car{ck}")
                nc.vector.reciprocal(t_r[:], t_e[:])          # sigmoid
                nc.vector.tensor_scalar_add(t_r[:], t_r[:], 1.0)  # 1+sigmoid
                t_s[ck] = t_r
            # z recompute at my positions: z = x @ Wz, silu via exp+recip
            t_z = {}
            for zi, (mod, ck) in enumerate(
                    (("vi", 0), ("vi", 1), ("ir", 0), ("ir", 1))):
                xt = t_xvc if mod == "vi" else t_xic
                p_z = pcp.tile([DM, PC], F32, tag="pz2")
                mmr(p_z[:], t_wz[:, zi, :], xt[:],
                    start=True, stop=True)
                t_e = pcq.tile([DM, PC], F32, tag="ze")
                nc.scalar.activation(t_e[:], p_z[:], AF.Exp,
                                     bias=0.0, scale=-1.0)
                nc.vector.tensor_scalar_add(t_e[:], t_e[:], 1.0)
                t_r = pcq.tile([DM, PC], F32, tag="zr")
                nc.vector.reciprocal(t_r[:], t_e[:])
                tz = pcq.tile([DM, PC], F32, tag=f"z{zi}", name=f"z{zi}")
                nc.vector.tensor_mul(tz[:], p_z[:], t_r[:])
                t_z[(mod, ck)] = tz
            # LN per modality
            t_fin = {}
            for mod in ("vi", "ir"):
                p_s1 = pcp.tile([1, PC], F32, tag="s1")
                p_s2 = pcp.tile([1, PC], F32, tag="s2")
                for ck in range(2):
                    nc.tensor.matmul(p_s1[:], t_onec[:],
                                     t_y[(mod, ck)][:], start=(ck == 0),
                                     stop=(ck == 1))
                for ck in range(2):
                    t_sq = pcq.tile([DM, PC], F32, tag="sq")
                    nc.scalar.activation(t_sq[:], t_y[(mod, ck)][:], AF.Square)
                    nc.tensor.matmul(p_s2[:], t_onec[:],
                                     t_sq[:], start=(ck == 0), stop=(ck == 1))
                t_mu = pcq.tile([1, PC], F32, tag="mu")
                nc.vector.tensor_scalar_mul(t_mu[:], p_s1[:], 1.0 / DI)
                t_musq = pcq.tile([1, PC], F32, tag="musq")
                nc.vector.tensor_mul(t_musq[:], t_mu[:], t_mu[:])
                t_var = pcq.tile([1, PC], F32, tag="var")
                nc.vector.scalar_tensor_tensor(t_var[:], p_s2[:], 1.0 / DI,
                                               t_musq[:], OP.mult, OP.subtract)
                t_eps = pcq.tile([1, 1], F32, tag="eps")
                nc.vector.memset(t_eps[:], 1e-5)
                t_lnv = pcq.tile([1, PC], F32, tag="lnv")
                nc.scalar.activation(t_lnv[:], t_var[:], AF.Ln,
                                     bias=t_eps[:], scale=1.0)
                t_rstd = pcq.tile([1, PC], F32, tag="rstd")
                nc.scalar.activation(t_rstd[:], t_lnv[:], AF.Exp,
                                     bias=0.0, scale=-0.5)
                t_mur = pcq.tile([1, PC], F32, tag="mur")
                nc.vector.tensor_mul(t_mur[:], t_mu[:], t_rstd[:])
                p_q = pcp.tile([DM, PC], F32, tag="pq")
                nc.tensor.matmul(p_q[:], t_oner[:], t_rstd[:],
                                 start=True, stop=True)
                p_m = pcp.tile([DM, PC], F32, tag="pm")
                nc.tensor.matmul(p_m[:], t_oner[:], t_mur[:],
                                 start=True, stop=True)
                gb = {"vi": (0, 1), "ir": (2, 3)}[mod]
                for ck in range(2):
                    t_t = pcq.tile([DM, PC], F32, tag="lt")
                    nc.vector.tensor_mul(t_t[:], t_y[(mod, ck)][:], p_q[:])
                    t_t2 = pcq.tile([DM, PC], F32, tag="lt2")
                    nc.vector.tensor_sub(t_t2[:], t_t[:], p_m[:])
                    t_yn = pcq.tile([DM, PC], F32, tag="yn")
                    nc.scalar.activation(t_yn[:], t_t2[:], AF.Identity,
                                         bias=t_lnw[:, ck, gb[1]:gb[1] + 1],
                                         scale=t_lnw[:, ck, gb[0]:gb[0] + 1])
                    # gate: fin += yn * z * s
                    t_m1 = pcq.tile([DM, PC], F32, tag="m1")
                    nc.vector.tensor_mul(t_m1[:], t_yn[:], t_z[(mod, ck)][:])
                    if mod == "vi":
                        t_f = pcq.tile([DM, PC], F32, tag=f"fin{ck}",
                                       name=f"fin{ck}")
                        nc.vector.tensor_scalar_mul(t_f[:].bitcast(F32R),
                                                    t_m1[:],
                                                    t_s[ck][:, 0:1])
                        t_fin[ck] = t_f
                    else:
                        nc.vector.scalar_tensor_tensor(
                            t_fin[ck][:].bitcast(F32R), t_m1[:],
                            t_s[ck][:, 1:2], t_fin[ck][:],
                            OP.mult, OP.add)
            p_o = pcp.tile([DM, PC], F32, tag="po")
            for ck in range(2):
                mmr(p_o[:], t_wout[:, ck, :], t_fin[ck][:],
                    start=(ck == 0), stop=(ck == 1))
            t_o = pcq.tile([DM, PC], F32, tag="o")
            nc.scalar.copy(t_o[:], p_o[:])
            nc.sync.dma_start(out=o_out[:], in_=t_o[:])

    nc.finalize()
    return nc


def _prep_inputs(inputs):
    """Host-side prep: slice/transpose weights per core. Returns in_maps."""
    g = {k: np.asarray(v, dtype=np.float32) for k, v in inputs.items()}
    x_vi = g["x_vi"].reshape(HW, DM)
    x_ir = g["x_ir"].reshape(HW, DM)
    xvt = np.ascontiguousarray(x_vi.T)
    xit = np.ascontiguousarray(x_ir.T)
    A = -np.exp(g["A_logs"]).reshape(K, DI, NST)
    Ds = g["Ds"].reshape(K, DI)
    in_maps = []
    for c in range(NCORES):
        S = slice(c * DSL, (c + 1) * DSL)
        m = {}
        m["xvt"], m["xit"] = xvt, xit
        w48v = np.zeros((DM, 64), np.float32)
        w48v[:, 0:DSL] = g["W_vi"][S].T
        w48v[:, 32:56] = g["W_vi"][DI:][S].T
        m["w48v"] = w48v
        w48i = np.zeros((DM, 64), np.float32)
        w48i[:, 0:DSL] = g["W_ir"][S].T
        w48i[:, 32:56] = g["W_ir"][DI:][S].T
        m["w48i"] = w48i
        m["wsub"] = np.ascontiguousarray(g["W_sub"][S].T)
        w72 = np.zeros((96, 9, 96), np.float32)
        b72 = np.zeros((96, 1), np.float32)
        for nm in ("sub", "vi", "ir"):
            o = XOFF[nm]
            cw = g[f"conv_w_{nm}"][S, 0]      # [DSL, 3, 3]
            for tap in range(9):
                for d in range(DSL):
                    w72[o + d, tap, o + d] = cw[d, tap // 3, tap % 3]
            b72[o:o + DSL, 0] = g[f"conv_b_{nm}"][S]
        m["w72"], m["b72"] = w72, b72
        # x_dbl lhsT per scan tile: blocks (tile, half) -> (k, src mod)
        w84 = np.zeros((96, 3, 28), np.float32)
        BLK = (((0, "sub"), (0, "vi")), ((1, "sub"), (1, "ir")),
               ((2, "vi"), (3, "ir")))
        for tg in range(3):
            for half, (k, nm) in enumerate(BLK[tg]):
                o = XOFF[nm]
                w84[o:o + DSL, tg, half * 14:(half + 1) * 14] = \
                    g["x_proj_weight"][k][:, S].T
        m["w84"] = w84
        wdtr = np.zeros((RNK, K, LANES), np.float32)
        dtb = np.zeros((LANES, K), np.float32)
        asc = np.zeros((LANES, K), np.float32)
        for k in range(K):
            for n in range(NST):
                for d in range(DSL):
                    lane = n * DSL + d
                    wdtr[:, k, lane] = g["dt_projs_weight"][k, c * DSL + d, :]
                    dtb[lane, k] = g["dt_projs_bias"][k, c * DSL + d]
                    asc[lane, k] = A[k, c * DSL + d, n]
        m["wdtr"], m["dtb"], m["asc"] = wdtr, dtb, asc
        m96 = np.zeros((LANES, DSL), np.float32)
        for n in range(NST):
            for d in range(DSL):
                m96[n * DSL + d, d] = 1
        m["m96"] = m96
        diagd = np.zeros((96, 2, DSL), np.float32)
        np.fill_diagonal(diagd[XOFF["vi"]:XOFF["vi"] + DSL, 0, :],
                         Ds[0, S] + Ds[2, S])
        np.fill_diagonal(diagd[XOFF["ir"]:XOFF["ir"] + DSL, 1, :],
                         Ds[1, S] + Ds[3, S])
        m["diagd"] = diagd
        f1 = np.zeros((DSL, 4, 12), np.float32)
        f1[:, 0] = g["ca_vi_f1"][:, S].T / HW
        f1[:, 1] = g["ca_vi_f1"][:, S].T
        f1[:, 2] = g["ca_ir_f1"][:, S].T / HW
        f1[:, 3] = g["ca_ir_f1"][:, S].T
        m["f1"] = f1
        f2 = np.zeros((12, 2, 2, DM), np.float32)
        for ck in range(2):
            f2[:, 0, ck] = g["ca_vi_f2"][ck * DM:(ck + 1) * DM].T
            f2[:, 1, ck] = g["ca_ir_f2"][ck * DM:(ck + 1) * DM].T
        m["f2"] = f2
        lnw = np.zeros((DM, 2, 4), np.float32)
        for ck in range(2):
            cs = slice(ck * DM, (ck + 1) * DM)
            lnw[:, ck, 0] = g["ln_vi_g"][cs]
            lnw[:, ck, 1] = g["ln_vi_b"][cs]
            lnw[:, ck, 2] = g["ln_ir_g"][cs]
            lnw[:, ck, 3] = g["ln_ir_b"][cs]
        m["lnw"] = lnw
        wout = np.zeros((DM, 2, DM), np.float32)
        for ck in range(2):
            wout[:, ck] = g["W_out"][:, ck * DM:(ck + 1) * DM].T
        m["wout"] = wout
        wz = np.zeros((DM, 4, DM), np.float32)
        wz[:, 0] = g["W_vi"][DI:][0:DM].T
        wz[:, 1] = g["W_vi"][DI:][DM:DI].T
        wz[:, 2] = g["W_ir"][DI:][0:DM].T
        wz[:, 3] = g["W_ir"][DI:][DM:DI].T
        m["wz"] = wz
        m["onec"] = np.ones((DM, 1), np.float32)
        m["oner"] = np.ones((1, DM), np.float32)
        m["xvc"] = np.ascontiguousarray(xvt[:, c * PC:(c + 1) * PC])
        m["xic"] = np.ascontiguousarray(xit[:, c * PC:(c + 1) * PC])
        in_maps.append(m)
    return in_maps


def kernel(**inputs):
    if "nc" not in _cache:
        _cache["nc"] = _build()
    nc = _cache["nc"]
    in_maps = _prep_inputs(inputs)
    res = run_bass_kernel_spmd(nc, in_maps, core_ids=list(range(NCORES)))
    out = np.zeros((DM, HW), np.float32)
    for c in range(NCORES):
        out[:, c * PC:(c + 1) * PC] = res.results[c]["out"]
    return out.T.reshape(B, H, W, DM).astype(np.float32)


# revision 25
# speedup vs baseline: 2.0765x; 1.0952x over previous
"""Trainium2 Bass kernel for the DSSM (dual-modality Mamba-style 2D selective
scan) module. 8-core SPMD: scan channels d-sharded (24/core x 4 directions),
upstream in_proj/dwconv d-sharded, downstream LN/out position-sharded.
Cross-core: 3 chunked AllReduces (x_dbl partials, one per scan tile,
overlapped with compute) + tiny v1 AllReduce + one AllToAll (y reshard).
"""
import sys
sys.path.insert(0, "/opt/trn_rl_repo")
import numpy as np
import ml_dtypes
import concourse.bass as bass
from concourse import mybir
from concourse.bacc import Bacc
from concourse.tile import TileContext
from concourse.tile_rust import add_dep_helper
from concourse.bass_utils import run_bass_kernel_spmd

F32 = mybir.dt.float32
F32R = mybir.dt.float32r
BF16 = mybir.dt.bfloat16
AF = mybir.ActivationFunctionType
OP = mybir.AluOpType

NCORES = 8
RG = [list(range(NCORES))]
B, H, W = 1, 48, 48
HW = H * W                      # 2304
L = 2 * HW                      # 4608
DM = 96                         # d_model
DI = 192                        # d_inner
NST = 4                         # d_state
RNK = 6                         # dt_rank
K = 4
DSL = DI // NCORES              # 24 channels per core
LANES = NST * DSL               # 96 scan lanes (lane = n*DSL + d)
CH = 512                        # phase-B PSUM column chunk
NCH = L // CH                   # 9
PC = HW // NCORES               # 288 positions per core (phase C)
RCH = 480                       # phase-A chunk = 10 image rows
ROWCHUNKS = [(0, 10), (10, 10), (20, 10), (30, 10), (40, 8)]
XOFF = {"sub": 0, "vi": 32, "ir": 64}   # row block in stacked xs96
# tile t: (half0 mod, half1 mod); k per segment = t (t<2) else 2/3
TMODS = (("sub", "vi"), ("sub", "ir"), ("vi", "ir"))

_cache = {}


def _build():
    nc = Bacc(trn_type="TRN2", num_devices=NCORES)
    EIn = dict(kind="ExternalInput")
    i_xvt = nc.dram_tensor("xvt", [DM, HW], F32, **EIn)
    i_xit = nc.dram_tensor("xit", [DM, HW], F32, **EIn)
    i_w48v = nc.dram_tensor("w48v", [DM, 64], F32, **EIn)  # [x@0|z@32] lhsT
    i_w48i = nc.dram_tensor("w48i", [DM, 64], F32, **EIn)  # [x@0|z@32]
    i_wsub = nc.dram_tensor("wsub", [DM, DSL], F32, **EIn)
    i_w72 = nc.dram_tensor("w72", [96, 9, 96], F32, **EIn)   # conv block-diag
    i_b72 = nc.dram_tensor("b72", [96, 1], F32, **EIn)       # conv bias stacked
    i_w84 = nc.dram_tensor("w84", [96, 3, 28], F32, **EIn)   # x_dbl per tile
    i_wdtr = nc.dram_tensor("wdtr", [RNK, K, LANES], F32, **EIn)
    i_dtb = nc.dram_tensor("dtb", [LANES, K], F32, **EIn)
    i_asc = nc.dram_tensor("asc", [LANES, K], F32, **EIn)
    i_m96 = nc.dram_tensor("m96", [LANES, DSL], F32, **EIn)
    i_diagd = nc.dram_tensor("diagd", [96, 2, DSL], F32, **EIn)  # (vi,ir) D
    i_f1 = nc.dram_tensor("f1", [DSL, 4, 12], F32, **EIn)   # (via,vim,ira,irm)
    i_f2 = nc.dram_tensor("f2", [12, 2, 2, DM], F32, **EIn)  # (mod, chunk, out)
    i_lnw = nc.dram_tensor("lnw", [DM, 2, 4], F32, **EIn)    # per chunk g/b
    i_wout = nc.dram_tensor("wout", [DM, 2, DM], F32, **EIn)
    i_wz = nc.dram_tensor("wz", [DM, 4, DM], F32, **EIn)     # z lhsT
    i_onec = nc.dram_tensor("onec", [DM, 1], F32, **EIn)
    i_oner = nc.dram_tensor("oner", [1, DM], F32, **EIn)
    i_xvc = nc.dram_tensor("xvc", [DM, PC], F32, **EIn)
    i_xic = nc.dram_tensor("xic", [DM, PC], F32, **EIn)
    o_out = nc.dram_tensor("out", [DM, PC], F32, kind="ExternalOutput")
    # collective DRAM buffers: per scan tile [half, row, col]
    d_ri = [nc.dram_tensor(f"d_ri{t}", [2, 14, HW], F32) for t in range(3)]
    d_ro = [nc.dram_tensor(f"d_ro{t}", [2, 14, HW], F32, addr_space="Shared")
            for t in range(3)]
    d_v1i = nc.dram_tensor("d_v1i", [12, 4], F32)
    d_v1o = nc.dram_tensor("d_v1o", [12, 4], F32, addr_space="Shared")
    d_a2i = nc.dram_tensor("d_a2i", [NCORES, 2 * DSL, PC], F32)
    d_a2o = nc.dram_tensor("d_a2o", [NCORES, 2 * DSL, PC], F32)

    def mmr(out, lhsT, rhs, **kw):
        # float32r matmul: 1 cycle/row (vs 4 for fp32) when free dim >= 256
        nc.tensor.matmul(out, lhsT.bitcast(F32R), rhs.bitcast(F32R), **kw)

    import contextlib
    with TileContext(nc) as tc, contextlib.ExitStack() as ctx:
        wpool = ctx.enter_context(tc.tile_pool(name="weights", bufs=1))
        big = ctx.enter_context(tc.tile_pool(name="big", bufs=1))

        def wtile(shape, src, rnd=False, dt=F32):
            t = wpool.tile(shape, dt, tag=src.name, name="w_" + src.name)
            if rnd:
                nc.sync.dma_start(out=t[:].bitcast(F32R),
                                  in_=src[:].bitcast(F32R))
            else:
                nc.sync.dma_start(out=t, in_=src[:])
            return t
        t_w48v = wtile([DM, 64], i_w48v, True)
        t_w48i = wtile([DM, 64], i_w48i, True)
        t_wsub = wtile([DM, DSL], i_wsub, True)
        t_w72 = wtile([96, 9, 96], i_w72, True)
        t_b72 = wtile([96, 1], i_b72)
        t_w84 = wtile([96, 3, 28], i_w84, True)
        t_wdtr = wtile([RNK, K, LANES], i_wdtr, True)
        t_dtb = wtile([LANES, K], i_dtb)
        t_asc = wtile([LANES, K], i_asc)
        t_m96 = wtile([LANES, DSL], i_m96, True)
        t_diagd = wtile([96, 2, DSL], i_diagd, True)
        t_f1 = wtile([DSL, 4, 12], i_f1)
        t_f2 = wtile([12, 2, 2, DM], i_f2)
        t_lnw = wtile([DM, 2, 4], i_lnw)
        t_wout = wtile([DM, 2, DM], i_wout, True)
        t_wz = wtile([DM, 4, DM], i_wz, True)
        t_onec = wtile([DM, 1], i_onec)
        t_oner = wtile([1, DM], i_oner)
        t_xvc = wtile([DM, PC], i_xvc, True)
        t_xic = wtile([DM, PC], i_xic, True)

        # persistent SBUF
        t_xs72 = big.tile([96, HW], F32, tag="xs72")   # (sub|vi|ir) @ 0/32/64
        t_yvi = big.tile([DSL, HW], F32, tag="yvi")
        t_yir = big.tile([DSL, HW], F32, tag="yir")

        # =========== PHASE A: upstream (d-sharded) ===========
        with tc.tile_pool(name="pa1", bufs=1) as pa1, \
             tc.tile_pool(name="pa", bufs=3) as pa, \
             tc.tile_pool(name="pap", bufs=2, space="PSUM") as pap, \
             tc.tile_pool(name="pas", bufs=2, space="PSUM") as pas, \
             tc.tile_pool(name="pav", bufs=2, space="PSUM") as pav, \
             tc.tile_pool(name="pav1", bufs=1, space="PSUM") as pav1:
            t_xvt = pa1.tile([DM, HW], F32, tag="xvt")
            nc.sync.dma_start(out=t_xvt[:].bitcast(F32R),
                              in_=i_xvt[:].bitcast(F32R))
            t_xit = pa1.tile([DM, HW], F32, tag="xit")
            nc.sync.dma_start(out=t_xit[:].bitcast(F32R),
                              in_=i_xit[:].bitcast(F32R))
            t_xdiff = pa1.tile([DM, HW], F32, tag="xdiff")
            nc.vector.tensor_sub(t_xdiff[:].bitcast(F32R), t_xvt[:], t_xit[:])

            pad72 = pa1.tile([96, 50, 50], F32, tag="pad72")
            nc.vector.memset(pad72[:], 0.0)

            # in_proj: merged [zv|xv] / [zi|xi] matmuls + sub
            t_zacc = pa1.tile([DSL, 2, len(ROWCHUNKS)], F32, tag="zacc")
            t_zc = {"vi": pa1.tile([DSL, HW], F32, tag="zcvi", name="zcvi"),
                    "ir": pa1.tile([DSL, HW], F32, tag="zcir", name="zcir")}
            for im, (mod, w48, xt) in enumerate(
                    (("vi", t_w48v, t_xvt), ("ir", t_w48i, t_xit))):
                for ic, (r0, nr) in enumerate(ROWCHUNKS):
                    cols = slice(r0 * W, (r0 + nr) * W)
                    p48 = pap.tile([64, RCH], F32, tag="p48")
                    mmr(p48[:, :nr * W], w48[:], xt[:, cols],
                        start=True, stop=True)
                    nc.scalar.activation(t_zc[mod][:, cols],
                                         p48[32:56, :nr * W], AF.Silu,
                                         accum_out=t_zacc[:, im, ic:ic + 1])
                    o = XOFF[mod]
                    nc.scalar.copy(
                        pad72[o:o + DSL, 1 + r0:1 + r0 + nr, 1:49]
                        .bitcast(F32R),
                        p48[0:DSL, :nr * W]
                        .rearrange("p (a b) -> p a b", a=nr))
            for (r0, nr) in ROWCHUNKS:
                cols = slice(r0 * W, (r0 + nr) * W)
                p24 = pas.tile([28, RCH], F32, tag="px")
                mmr(p24[0:DSL, :nr * W], t_wsub[:], t_xdiff[:, cols],
                    start=True, stop=True)
                nc.scalar.copy(
                    pad72[0:DSL, 1 + r0:1 + r0 + nr, 1:49].bitcast(F32R),
                    p24[0:DSL, :nr * W].rearrange("p (a b) -> p a b", a=nr))

            # chan-attn pooled stats -> v1 partials -> tiny AR
            t_pool = pa1.tile([DSL, 4], F32, tag="tpool")
            nc.vector.tensor_reduce(t_pool[:, 0:1], t_zacc[:, 0, :],
                                    axis=mybir.AxisListType.X, op=OP.add)
            nc.vector.tensor_reduce(t_pool[:, 1:2], t_zc["vi"][:],
                                    axis=mybir.AxisListType.X, op=OP.max)
            nc.vector.tensor_reduce(t_pool[:, 2:3], t_zacc[:, 1, :],
                                    axis=mybir.AxisListType.X, op=OP.add)
            nc.vector.tensor_reduce(t_pool[:, 3:4], t_zc["ir"][:],
                                    axis=mybir.AxisListType.X, op=OP.max)
            t_v1 = pa1.tile([12, 4], F32, tag="tv1")
            p_v1 = pav1.tile([12, 4], F32, tag="pv1")
            for j in range(4):
                nc.tensor.matmul(p_v1[:, j:j + 1], t_f1[:, j, :],
                                 t_pool[:, j:j + 1], start=True, stop=True)
            nc.scalar.copy(t_v1[:], p_v1[:])
            nc.sync.dma_start(out=d_v1i[:], in_=t_v1[:])

            # depthwise conv 3x3: block-diag 72-channel, 9 taps
            for (r0, nr) in ROWCHUNKS:
                p_c = pav.tile([96, RCH], F32, tag="pconv")
                for tap in range(9):
                    dy, dx = tap // 3, tap % 3
                    mmr(p_c[:, :nr * W], t_w72[:, tap, :],
                        pad72[:, r0 + dy:r0 + dy + nr, dx:dx + 48],
                        start=(tap == 0), stop=(tap == 8))
                nc.scalar.activation(
                    t_xs72[:, r0 * W:(r0 + nr) * W].bitcast(F32R),
                    p_c[:, :nr * W], AF.Silu, bias=t_b72[:, 0:1], scale=1.0)

            # x_dbl partials per scan tile -> DRAM -> chunked AllReduce
            ar_inst = [None, None, None]
            for tg in range(3):
                for (r0, nr) in ROWCHUNKS:
                    cols = slice(r0 * W, (r0 + nr) * W)
                    p84 = pas.tile([28, RCH], F32, tag="px")
                    mmr(p84[:, :nr * W], t_w84[:, tg, :],
                        t_xs72[:, cols], start=True, stop=True)
                    t_xe = pa.tile([28, RCH], F32, tag="txdbl", name="t_xe")
                    nc.scalar.copy(t_xe[:, :nr * W], p84[:, :nr * W])
                    nc.sync.dma_start(
                        out=d_ri[tg][:, :, r0 * W:(r0 + nr) * W],
                        in_=t_xe[:, :nr * W]
                        .rearrange("(h p) c -> h p c", h=2))
                ar_inst[tg] = nc.gpsimd.collective_compute(
                    "AllReduce", OP.add, RG,
                    ins=[d_ri[tg][:]], outs=[d_ro[tg][:]])
            ar_v1 = nc.gpsimd.collective_compute(
                "AllReduce", OP.add, RG, ins=[d_v1i[:]], outs=[d_v1o[:]])

        # =========== PHASE B: scan middle (full-tile staging) ===========
        with tc.tile_pool(name="pb", bufs=1) as pb, \
             tc.tile_pool(name="pb2", bufs=2) as pb2, \
             tc.tile_pool(name="pbp", bufs=2, space="PSUM") as pbp, \
             tc.tile_pool(name="pby", bufs=2, space="PSUM") as pby:
            for t in range(3):
                segs = ([(0, L, t)] if t < 2 else
                        [(0, HW, 2), (HW, L, 3)])  # (start, end, k) tile cols
                yc0 = HW if t < 2 else 0            # y column span
                dro = d_ro[t]
                # ---- full-tile staged loads (DMA) ----
                t_rR = pb2.tile([RNK, L], F32, tag="rR")
                ld = nc.sync.dma_start(
                    out=t_rR[:].rearrange("p (h c) -> p h c", h=2)
                    .bitcast(F32R),
                    in_=dro[:, 0:RNK, :].transpose([1, 0, 2]).bitcast(F32R))
                add_dep_helper(ld.ins, ar_inst[t].ins, reason="rR after AR")
                # B replicated to lanes (lane = n*DSL + d) via broadcast DMA
                t_Brep = pb.tile([LANES, L], F32, tag="Brep")
                for half in (0, 1):
                    ld = nc.sync.dma_start(
                        out=t_Brep[:, half * HW:(half + 1) * HW],
                        in_=dro[half, RNK:RNK + NST, :]
                        .unsqueeze(1).broadcast_to([NST, DSL, HW]))
                    add_dep_helper(ld.ins, ar_inst[t].ins,
                                   reason="Brep after AR")
                # xs replicated to lanes (4 plain copies per half)
                t_urep = pb.tile([LANES, L], F32, tag="urep")
                for half in (0, 1):
                    o = XOFF[TMODS[t][half]]
                    for n in range(NST):
                        nc.sync.dma_start(
                            out=t_urep[n * DSL:(n + 1) * DSL,
                                       half * HW:(half + 1) * HW],
                            in_=t_xs72[o:o + DSL, :])
                # C replicated (y cols only)
                t_Crep = pb.tile([LANES, L], F32, tag="Crep")
                for half in ((1,) if t < 2 else (0, 1)):
                    ld = nc.sync.dma_start(
                        out=t_Crep[:, half * HW:(half + 1) * HW],
                        in_=dro[half, RNK + NST:14, :]
                        .unsqueeze(1).broadcast_to([NST, DSL, HW]))
                    add_dep_helper(ld.ins, ar_inst[t].ins,
                                   reason="Crep after AR")

                # ---- dts chunks: matmul + Exp evac (exp table) ----
                t_et = pb2.tile([LANES, L], F32, tag="eta")
                for c in range(NCH):
                    c0 = c * CH
                    pieces = [(max(s, c0), min(e, c0 + CH), k)
                              for (s, e, k) in segs
                              if e > c0 and s < c0 + CH]
                    p_dts = pbp.tile([LANES, CH], F32, tag="dts")
                    for (s, e, k) in pieces:
                        mmr(p_dts[:, s - c0:e - c0], t_wdtr[:, k, :],
                            t_rR[:, s:e], start=True, stop=True)
                    for (s, e, k) in pieces:
                        nc.scalar.activation(t_et[:, s:e],
                                             p_dts[:, s - c0:e - c0], AF.Exp,
                                             bias=t_dtb[:, k:k + 1], scale=1.0)
                # ---- delta = softplus (ln table), then a = exp(asc*delta) ----
                t_delta = pb.tile([LANES, L], F32, tag="delta")
                nc.scalar.activation(t_delta[:], t_et[:], AF.Ln,
                                     bias=1.0, scale=1.0)
                t_a = pb2.tile([LANES, L], F32, tag="eta")  # reuse et buffer
                for (s, e, k) in segs:
                    nc.scalar.activation(t_a[:, s:e], t_delta[:, s:e], AF.Exp,
                                         bias=0.0, scale=t_asc[:, k:k + 1])
                # ---- b = delta * B_rep * xs_rep (in-place, DVE + gpsimd) ----
                SPL = 3072   # DVE gets 2/3, gpsimd 1/3 (eff ~0.42)
                nc.vector.tensor_mul(t_Brep[:, 0:SPL], t_delta[:, 0:SPL],
                                     t_Brep[:, 0:SPL])
                nc.gpsimd.tensor_mul(t_Brep[:, SPL:L], t_delta[:, SPL:L],
                                     t_Brep[:, SPL:L])
                nc.vector.tensor_mul(t_Brep[:, 0:SPL], t_Brep[:, 0:SPL],
                                     t_urep[:, 0:SPL])
                nc.gpsimd.tensor_mul(t_Brep[:, SPL:L], t_Brep[:, SPL:L],
                                     t_urep[:, SPL:L])
                # ---- scan ----
                t_h = pb.tile([LANES, L], F32, tag="h")
                if t < 2:
                    nc.vector.tensor_tensor_scan(t_h[:].bitcast(F32R), t_a[:],
                                                 t_Brep[:], 0.0,
                                                 OP.mult, OP.add)
                else:
                    for (s, e, k) in segs:   # reversed scans, fresh state
                        nc.vector.tensor_tensor_scan(
                            t_h[:, s:e][:, ::-1].bitcast(F32R),
                            t_a[:, s:e][:, ::-1],
                            t_Brep[:, s:e][:, ::-1], 0.0, OP.mult, OP.add)
                # ---- hc = h * C_rep (in-place into h), y cols only ----
                nc.gpsimd.tensor_mul(t_h[:, yc0:L].bitcast(F32R),
                                     t_h[:, yc0:L], t_Crep[:, yc0:L])
                # ---- y = m96 @ hc (+ D skip) -> accumulate into yvi/yir ----
                c0 = yc0
                while c0 < L:
                    c1 = min(c0 + CH, L)
                    p_y = pby.tile([DSL, CH], F32, tag="y")
                    mmr(p_y[:, :c1 - c0], t_m96[:], t_h[:, c0:c1],
                        start=True, stop=(t == 2))
                    if t < 2:   # D-skip, combined (D_k + D_{k+2}) on fwd tiles
                        o = XOFF[TMODS[t][1]]
                        mmr(p_y[:, :c1 - c0], t_diagd[o:o + DSL, t, :],
                            t_xs72[o:o + DSL, c0 - HW:c1 - HW],
                            start=False, stop=True)
                        dst = t_yvi if t == 0 else t_yir
                        nc.scalar.copy(dst[:, c0 - HW:c1 - HW],
                                       p_y[:, :c1 - c0])
                    else:
                        for (s, e) in ((c0, min(c1, HW)), (max(c0, HW), c1)):
                            if e <= s:
                                continue
                            if e <= HW:
                                nc.vector.tensor_add(
                                    t_yvi[:, s:e], t_yvi[:, s:e],
                                    p_y[:, s - c0:e - c0])
                            else:
                                nc.vector.tensor_add(
                                    t_yir[:, s - HW:e - HW],
                                    t_yir[:, s - HW:e - HW],
                                    p_y[:, s - c0:e - c0])
                    c0 = c1

        # =========== A2A: reshard y channels -> positions ===========
        for j in range(NCORES):
            nc.sync.dma_start(out=d_a2i[j, 0:DSL, :],
                              in_=t_yvi[:, j * PC:(j + 1) * PC])
            nc.sync.dma_start(out=d_a2i[j, DSL:2 * DSL, :],
                              in_=t_yir[:, j * PC:(j + 1) * PC])
        a2a_inst = nc.gpsimd.collective_compute(
            "AllToAll", OP.bypass, RG, ins=[d_a2i[:]], outs=[d_a2o[:]])
        t_v1o = big.tile([12, 4], F32, tag="v1o")
        ld = nc.sync.dma_start(out=t_v1o[:], in_=d_v1o[:])
        add_dep_helper(ld.ins, ar_v1.ins, reason="v1 after AR")

        # =========== PHASE C: LN + gate + out (position-sharded) ===========
        with tc.tile_pool(name="pcq", bufs=2) as pcq, \
             tc.tile_pool(name="pcp", bufs=1, space="PSUM") as pcp:
            # gather y chunks [96, PC] x (2 chunks, 2 mods)
            t_y = {}
            for mod, roff in (("vi", 0), ("ir", DSL)):
                for ck in range(2):
                    ty = pcq.tile([DM, PC], F32, tag=f"y{mod}{ck}",
                                  name=f"y{mod}{ck}")
                    for jj in range(4):
                        j = ck * 4 + jj
                        ld = nc.sync.dma_start(
                            out=ty[jj * DSL:(jj + 1) * DSL, :].bitcast(F32R),
                            in_=d_a2o[j, roff:roff + DSL, :].bitcast(F32R))
                        add_dep_helper(ld.ins, a2a_inst.ins,
                                       reason="y after A2A")
                    t_y[(mod, ck)] = ty
            # chan-attn scales s = 1 + sigmoid(f2 @ (relu(va)+relu(vm)))
            t_vr = pcq.tile([12, 4], F32, tag="vr")
            nc.scalar.activation(t_vr[:], t_v1o[:], AF.Relu)
            t_vw = pcq.tile([12, 2], F32, tag="vw")
            nc.vector.tensor_add(t_vw[:, 0:1], t_vr[:, 0:1], t_vr[:, 1:2])
            nc.vector.tensor_add(t_vw[:, 1:2], t_vr[:, 2:3], t_vr[:, 3:4])
            t_s = {}
            for ck in range(2):
                p_ca = pcp.tile([DM, 2], F32, tag="pca")
                for mod_i in range(2):
                    nc.tensor.matmul(p_ca[:, mod_i:mod_i + 1],
                                     t_f2[:, mod_i, ck, :],
                                     t_vw[:, mod_i:mod_i + 1],
                                     start=True, stop=True)
                t_e = pcq.tile([DM, 2], F32, tag="cae")
                nc.scalar.activation(t_e[:], p_ca[:], AF.Exp,
                                     bias=0.0, scale=-1.0)
                nc.vector.tensor_scalar_add(t_e[:], t_e[:], 1.0)
                t_r = pcq.tile([DM, 2], F32, tag=f"car{ck}", name=f"car{ck}")
                nc.vector.reciprocal(t_r[:], t_e[:])          # sigmoid
                nc.vector.tensor_scalar_add(t_r[:], t_r[:], 1.0)  # 1+sigmoid
                t_s[ck] = t_r
            # z recompute at my positions: z = x @ Wz, silu via exp+recip
            t_z = {}
            for zi, (mod, ck) in enumerate(
                    (("vi", 0), ("vi", 1), ("ir", 0), ("ir", 1))):
                xt = t_xvc if mod == "vi" else t_xic
                p_z = pcp.tile([DM, PC], F32, tag="pz2")
                mmr(p_z[:], t_wz[:, zi, :], xt[:],
                    start=True, stop=True)
                t_e = pcq.tile([DM, PC], F32, tag="ze")
                nc.scalar.activation(t_e[:], p_z[:], AF.Exp,
                                     bias=0.0, scale=-1.0)
                nc.vector.tensor_scalar_add(t_e[:], t_e[:], 1.0)
                t_r = pcq.tile([DM, PC], F32, tag="zr")
                nc.vector.reciprocal(t_r[:], t_e[:])
                tz = pcq.tile([DM, PC], F32, tag=f"z{zi}", name=f"z{zi}")
                nc.vector.tensor_mul(tz[:], p_z[:], t_r[:])
                t_z[(mod, ck)] = tz
            # LN per modality
            t_fin = {}
            for mod in ("vi", "ir"):
                p_s1 = pcp.tile([1, PC], F32, tag="s1")
                p_s2 = pcp.tile([1, PC], F32, tag="s2")
                for ck in range(2):
                    nc.tensor.matmul(p_s1[:], t_onec[:],
                                     t_y[(mod, ck)][:], start=(ck == 0),
                                     stop=(ck == 1))
                for ck in range(2):
                    t_sq = pcq.tile([DM, PC], F32, tag="sq")
                    nc.scalar.activation(t_sq[:], t_y[(mod, ck)][:], AF.Square)
                    nc.tensor.matmul(p_s2[:], t_onec[:],
                                     t_sq[:], start=(ck == 0), stop=(ck == 1))
                t_mu = pcq.tile([1, PC], F32, tag="mu")
                nc.vector.tensor_scalar_mul(t_mu[:], p_s1[:], 1.0 / DI)
                t_musq = pcq.tile([1, PC], F32, tag="musq")
                nc.vector.tensor_mul(t_musq[:], t_mu[:], t_mu[:])
                t_var = pcq.tile([1, PC], F32, tag="var")
                nc.vector.scalar_tensor_tensor(t_var[:], p_s2[:], 1.0 / DI,
                                               t_musq[:], OP.mult, OP.subtract)
                t_eps = pcq.tile([1, 1], F32, tag="eps")
                nc.vector.memset(t_eps[:], 1e-5)
                t_lnv = pcq.tile([1, PC], F32, tag="lnv")
                nc.scalar.activation(t_lnv[:], t_var[:], AF.Ln,
                                     bias=t_eps[:], scale=1.0)
                t_rstd = pcq.tile([1, PC], F32, tag="rstd")
                nc.scalar.activation(t_rstd[:], t_lnv[:], AF.Exp,
                                     bias=0.0, scale=-0.5)
                t_mur = pcq.tile([1, PC], F32, tag="mur")
                nc.vector.tensor_mul(t_mur[:], t_mu[:], t_rstd[:])
                p_q = pcp.tile([DM, PC], F32, tag="pq")
                nc.tensor.matmul(p_q[:], t_oner[:], t_rstd[:],
                                 start=True, stop=True)
                p_m = pcp.tile([DM, PC], F32, tag="pm")
                nc.tensor.matmul(p_m[:], t_oner[:], t_mur[:],
                                 start=True, stop=True)
                gb = {"vi": (0, 1), "ir": (2, 3)}[mod]
                for ck in range(2):
                    t_t = pcq.tile([DM, PC], F32, tag="lt")
                    nc.vector.tensor_mul(t_t[:], t_y[(mod, ck)][:], p_q[:])
                    t_t2 = pcq.tile([DM, PC], F32, tag="lt2")
                    nc.vector.tensor_sub(t_t2[:], t_t[:], p_m[:])
                    t_yn = pcq.tile([DM, PC], F32, tag="yn")
                    nc.scalar.activation(t_yn[:], t_t2[:], AF.Identity,
                                         bias=t_lnw[:, ck, gb[1]:gb[1] + 1],
                                         scale=t_lnw[:, ck, gb[0]:gb[0] + 1])
                    # gate: fin += yn * z * s
                    t_m1 = pcq.tile([DM, PC], F32, tag="m1")
                    nc.vector.tensor_mul(t_m1[:], t_yn[:], t_z[(mod, ck)][:])
                    if mod == "vi":
                        t_f = pcq.tile([DM, PC], F32, tag=f"fin{ck}",
                                       name=f"fin{ck}")
                        nc.vector.tensor_scalar_mul(t_f[:].bitcast(F32R),
                                                    t_m1[:],
                                                    t_s[ck][:, 0:1])
                        t_fin[ck] = t_f
                    else:
                        nc.vector.scalar_tensor_tensor(
                            t_fin[ck][:].bitcast(F32R), t_m1[:],
                            t_s[ck][:, 1:2], t_fin[ck][:],
                            OP.mult, OP.add)
            p_o = pcp.tile([DM, PC], F32, tag="po")
            for ck in range(2):
                mmr(p_o[:], t_wout[:, ck, :], t_fin[ck][:],
                    start=(ck == 0), stop=(ck == 1))
            t_o = pcq.tile([DM, PC], F32, tag="o")
            nc.scalar.copy(t_o[:], p_o[:])
            nc.sync.dma_start(out=o_out[:], in_=t_o[:])

    nc.finalize()
    return nc


def _prep_inputs(inputs):
    """Host-side prep: slice/transpose weights per core. Returns in_maps."""
    g = {k: np.asarray(v, dtype=np.float32) for k, v in inputs.items()}
    x_vi = g["x_vi"].reshape(HW, DM)
    x_ir = g["x_ir"].reshape(HW, DM)
    xvt = np.ascontiguousarray(x_vi.T)
    xit = np.ascontiguousarray(x_ir.T)
    A = -np.exp(g["A_logs"]).reshape(K, DI, NST)
    Ds = g["Ds"].reshape(K, DI)
    in_maps = []
    for c in range(NCORES):
        S = slice(c * DSL, (c + 1) * DSL)
        m = {}
        m["xvt"], m["xit"] = xvt, xit
        w48v = np.zeros((DM, 64), np.float32)
        w48v[:, 0:DSL] = g["W_vi"][S].T
        w48v[:, 32:56] = g["W_vi"][DI:][S].T
        m["w48v"] = w48v
        w48i = np.zeros((DM, 64), np.float32)
        w48i[:, 0:DSL] = g["W_ir"][S].T
        w48i[:, 32:56] = g["W_ir"][DI:][S].T
        m["w48i"] = w48i
        m["wsub"] = np.ascontiguousarray(g["W_sub"][S].T)
        w72 = np.zeros((96, 9, 96), np.float32)
        b72 = np.zeros((96, 1), np.float32)
        for nm in ("sub", "vi", "ir"):
            o = XOFF[nm]
            cw = g[f"conv_w_{nm}"][S, 0]      # [DSL, 3, 3]
            for tap in range(9):
                for d in range(DSL):
                    w72[o + d, tap, o + d] = cw[d, tap // 3, tap % 3]
            b72[o:o + DSL, 0] = g[f"conv_b_{nm}"][S]
        m["w72"], m["b72"] = w72, b72
        # x_dbl lhsT per scan tile: blocks (tile, half) -> (k, src mod)
        w84 = np.zeros((96, 3, 28), np.float32)
        BLK = (((0, "sub"), (0, "vi")), ((1, "sub"), (1, "ir")),
               ((2, "vi"), (3, "ir")))
        for tg in range(3):
            for half, (k, nm) in enumerate(BLK[tg]):
                o = XOFF[nm]
                w84[o:o + DSL, tg, half * 14:(half + 1) * 14] = \
                    g["x_proj_weight"][k][:, S].T
        m["w84"] = w84
        wdtr = np.zeros((RNK, K, LANES), np.float32)
        dtb = np.zeros((LANES, K), np.float32)
        asc = np.zeros((LANES, K), np.float32)
        for k in range(K):
            for n in range(NST):
                for d in range(DSL):
                    lane = n * DSL + d
                    wdtr[:, k, lane] = g["dt_projs_weight"][k, c * DSL + d, :]
                    dtb[lane, k] = g["dt_projs_bias"][k, c * DSL + d]
                    asc[lane, k] = A[k, c * DSL + d, n]
        m["wdtr"], m["dtb"], m["asc"] = wdtr, dtb, asc
        m96 = np.zeros((LANES, DSL), np.float32)
        for n in range(NST):
            for d in range(DSL):
                m96[n * DSL + d, d] = 1
        m["m96"] = m96
        diagd = np.zeros((96, 2, DSL), np.float32)
        np.fill_diagonal(diagd[XOFF["vi"]:XOFF["vi"] + DSL, 0, :],
                         Ds[0, S] + Ds[2, S])
        np.fill_diagonal(diagd[XOFF["ir"]:XOFF["ir"] + DSL, 1, :],
                         Ds[1, S] + Ds[3, S])
        m["diagd"] = diagd
        f1 = np.zeros((DSL, 4, 12), np.float32)
        f1[:, 0] = g["ca_vi_f1"][:, S].T / HW
        f1[:, 1] = g["ca_vi_f1"][:, S].T
        f1[:, 2] = g["ca_ir_f1"][:, S].T / HW
        f1[:, 3] = g["ca_ir_f1"][:, S].T
        m["f1"] = f1
        f2 = np.zeros((12, 2, 2, DM), np.float32)
        for ck in range(2):
            f2[:, 0, ck] = g["ca_vi_f2"][ck * DM:(ck + 1) * DM].T
            f2[:, 1, ck] = g["ca_ir_f2"][ck * DM:(ck + 1) * DM].T
        m["f2"] = f2
        lnw = np.zeros((DM, 2, 4), np.float32)
        for ck in range(2):
            cs = slice(ck * DM, (ck + 1) * DM)
            lnw[:, ck, 0] = g["ln_vi_g"][cs]
            lnw[:, ck, 1] = g["ln_vi_b"][cs]
            lnw[:, ck, 2] = g["ln_ir_g"][cs]
            lnw[:, ck, 3] = g["ln_ir_b"][cs]
        m["lnw"] = lnw
        wout = np.zeros((DM, 2, DM), np.float32)
        for ck in range(2):
            wout[:, ck] = g["W_out"][:, ck * DM:(ck + 1) * DM].T
        m["wout"] = wout
        wz = np.zeros((DM, 4, DM), np.float32)
        wz[:, 0] = g["W_vi"][DI:][0:DM].T
        wz[:, 1] = g["W_vi"][DI:][DM:DI].T
        wz[:, 2] = g["W_ir"][DI:][0:DM].T
        wz[:, 3] = g["W_ir"][DI:][DM:DI].T
        m["wz"] = wz
        m["onec"] = np.ones((DM, 1), np.float32)
        m["oner"] = np.ones((1, DM), np.float32)
        m["xvc"] = np.ascontiguousarray(xvt[:, c * PC:(c + 1) * PC])
        m["xic"] = np.ascontiguousarray(xit[:, c * PC:(c + 1) * PC])
        in_maps.append(m)
    return in_maps


def kernel(**inputs):
    if "nc" not in _cache:
        _cache["nc"] = _build()
    nc = _cache["nc"]
    in_maps = _prep_inputs(inputs)
    res = run_bass_kernel_spmd(nc, in_maps, core_ids=list(range(NCORES)))
    out = np.zeros((DM, HW), np.float32)
    for c in range(NCORES):
        out[:, c * PC:(c + 1) * PC] = res.results[c]["out"]
    return out.T.reshape(B, H, W, DM).astype(np.float32)


# revision 27
# speedup vs baseline: 2.2774x; 1.0967x over previous
"""Trainium2 Bass kernel for the DSSM (dual-modality Mamba-style 2D selective
scan) module. 8-core SPMD: scan channels d-sharded (24/core x 4 directions),
upstream in_proj/dwconv d-sharded, downstream LN/out position-sharded.
Cross-core: 3 chunked AllReduces (x_dbl partials, one per scan tile,
overlapped with compute) + tiny v1 AllReduce + one AllToAll (y reshard).
"""
import sys
sys.path.insert(0, "/opt/trn_rl_repo")
import numpy as np
import ml_dtypes
import concourse.bass as bass
from concourse import mybir
from concourse.bacc import Bacc
from concourse.tile import TileContext
from concourse.tile_rust import add_dep_helper
from concourse.bass_utils import run_bass_kernel_spmd

F32 = mybir.dt.float32
F32R = mybir.dt.float32r
BF16 = mybir.dt.bfloat16
AF = mybir.ActivationFunctionType
OP = mybir.AluOpType

NCORES = 8
RG = [list(range(NCORES))]
B, H, W = 1, 48, 48
HW = H * W                      # 2304
L = 2 * HW                      # 4608
DM = 96                         # d_model
DI = 192                        # d_inner
NST = 4                         # d_state
RNK = 6                         # dt_rank
K = 4
DSL = DI // NCORES              # 24 channels per core
LANES = NST * DSL               # 96 scan lanes (lane = n*DSL + d)
CH = 512                        # phase-B PSUM column chunk
NCH = L // CH                   # 9
PC = HW // NCORES               # 288 positions per core (phase C)
RCH = 480                       # phase-A chunk = 10 image rows
ROWCHUNKS = [(0, 10), (10, 10), (20, 10), (30, 10), (40, 8)]
XOFF = {"sub": 0, "vi": 32, "ir": 64}   # row block in stacked xs96
# tile t: (half0 mod, half1 mod); k per segment = t (t<2) else 2/3
TMODS = (("sub", "vi"), ("sub", "ir"), ("vi", "ir"))

_cache = {}


def _build():
    nc = Bacc(trn_type="TRN2", num_devices=NCORES)
    EIn = dict(kind="ExternalInput")
    i_xvt = nc.dram_tensor("xvt", [DM, HW], F32, **EIn)
    i_xit = nc.dram_tensor("xit", [DM, HW], F32, **EIn)
    i_w48v = nc.dram_tensor("w48v", [DM, 64], F32, **EIn)  # [x@0|z@32] lhsT
    i_w48i = nc.dram_tensor("w48i", [DM, 64], F32, **EIn)  # [x@0|z@32]
    i_wsub = nc.dram_tensor("wsub", [DM, DSL], F32, **EIn)
    i_w72 = nc.dram_tensor("w72", [96, 9, 96], F32, **EIn)   # conv block-diag
    i_b72 = nc.dram_tensor("b72", [96, 1], F32, **EIn)       # conv bias stacked
    i_w84 = nc.dram_tensor("w84", [96, 3, 28], BF16, **EIn)  # x_dbl per tile
    i_wdtr = nc.dram_tensor("wdtr", [RNK, K, LANES], BF16, **EIn)
    i_dtb = nc.dram_tensor("dtb", [LANES, K], F32, **EIn)
    i_asc = nc.dram_tensor("asc", [LANES, K], F32, **EIn)
    i_m96 = nc.dram_tensor("m96", [LANES, DSL], F32, **EIn)
    i_diagd = nc.dram_tensor("diagd", [96, 2, DSL], BF16, **EIn)  # (vi,ir) D
    i_f1 = nc.dram_tensor("f1", [DSL, 4, 12], F32, **EIn)   # (via,vim,ira,irm)
    i_f2 = nc.dram_tensor("f2", [12, 2, 2, DM], F32, **EIn)  # (mod, chunk, out)
    i_lnw = nc.dram_tensor("lnw", [DM, 2, 4], F32, **EIn)    # per chunk g/b
    i_wout = nc.dram_tensor("wout", [DM, 2, DM], F32, **EIn)
    i_wz = nc.dram_tensor("wz", [DM, 4, DM], F32, **EIn)     # z lhsT
    i_onec = nc.dram_tensor("onec", [DM, 1], F32, **EIn)
    i_oner = nc.dram_tensor("oner", [1, DM], F32, **EIn)
    i_xvc = nc.dram_tensor("xvc", [DM, PC], F32, **EIn)
    i_xic = nc.dram_tensor("xic", [DM, PC], F32, **EIn)
    o_out = nc.dram_tensor("out", [DM, PC], F32, kind="ExternalOutput")
    # collective DRAM buffers: per scan tile [half, row, col]
    d_ri = [nc.dram_tensor(f"d_ri{t}", [2, 14, HW], BF16) for t in range(3)]
    d_ro = [nc.dram_tensor(f"d_ro{t}", [2, 14, HW], BF16, addr_space="Shared")
            for t in range(3)]
    d_v1i = nc.dram_tensor("d_v1i", [12, 4], F32)
    d_v1o = nc.dram_tensor("d_v1o", [12, 4], F32, addr_space="Shared")
    d_a2i = nc.dram_tensor("d_a2i", [NCORES, 2 * DSL, PC], F32)
    d_a2o = nc.dram_tensor("d_a2o", [NCORES, 2 * DSL, PC], F32)

    def mmr(out, lhsT, rhs, **kw):
        # float32r matmul: 1 cycle/row (vs 4 for fp32) when free dim >= 256
        nc.tensor.matmul(out, lhsT.bitcast(F32R), rhs.bitcast(F32R), **kw)

    import contextlib
    with TileContext(nc) as tc, contextlib.ExitStack() as ctx:
        wpool = ctx.enter_context(tc.tile_pool(name="weights", bufs=1))
        big = ctx.enter_context(tc.tile_pool(name="big", bufs=1))

        def wtile(shape, src, rnd=False, dt=F32):
            t = wpool.tile(shape, dt, tag=src.name, name="w_" + src.name)
            if rnd:
                nc.sync.dma_start(out=t[:].bitcast(F32R),
                                  in_=src[:].bitcast(F32R))
            else:
                nc.sync.dma_start(out=t, in_=src[:])
            return t
        t_w48v = wtile([DM, 64], i_w48v, True)
        t_w48i = wtile([DM, 64], i_w48i, True)
        t_wsub = wtile([DM, DSL], i_wsub, True)
        t_w72 = wtile([96, 9, 96], i_w72, True)
        t_b72 = wtile([96, 1], i_b72)
        t_w84 = wtile([96, 3, 28], i_w84, dt=BF16)
        t_wdtr = wtile([RNK, K, LANES], i_wdtr, dt=BF16)
        t_dtb = wtile([LANES, K], i_dtb)
        t_asc = wtile([LANES, K], i_asc)
        t_m96 = wtile([LANES, DSL], i_m96, True)
        t_diagd = wtile([96, 2, DSL], i_diagd, dt=BF16)
        t_f1 = wtile([DSL, 4, 12], i_f1)
        t_f2 = wtile([12, 2, 2, DM], i_f2)
        t_lnw = wtile([DM, 2, 4], i_lnw)
        t_wout = wtile([DM, 2, DM], i_wout, True)
        t_wz = wtile([DM, 4, DM], i_wz, True)
        t_onec = wtile([DM, 1], i_onec)
        t_oner = wtile([1, DM], i_oner)
        t_xvc = wtile([DM, PC], i_xvc, True)
        t_xic = wtile([DM, PC], i_xic, True)

        # persistent SBUF
        t_xs72 = big.tile([96, HW], BF16, tag="xs72")  # (sub|vi|ir) @ 0/32/64
        t_yvi = big.tile([DSL, HW], F32, tag="yvi")
        t_yir = big.tile([DSL, HW], F32, tag="yir")

        # =========== PHASE A: upstream (d-sharded) ===========
        with tc.tile_pool(name="pa1", bufs=1) as pa1, \
             tc.tile_pool(name="pa", bufs=3) as pa, \
             tc.tile_pool(name="pap", bufs=2, space="PSUM") as pap, \
             tc.tile_pool(name="pas", bufs=2, space="PSUM") as pas, \
             tc.tile_pool(name="pav", bufs=2, space="PSUM") as pav, \
             tc.tile_pool(name="pav1", bufs=1, space="PSUM") as pav1:
            t_xvt = pa1.tile([DM, HW], F32, tag="xvt")
            nc.sync.dma_start(out=t_xvt[:].bitcast(F32R),
                              in_=i_xvt[:].bitcast(F32R))
            t_xit = pa1.tile([DM, HW], F32, tag="xit")
            nc.sync.dma_start(out=t_xit[:].bitcast(F32R),
                              in_=i_xit[:].bitcast(F32R))
            t_xdiff = pa1.tile([DM, HW], F32, tag="xdiff")
            nc.vector.tensor_sub(t_xdiff[:].bitcast(F32R), t_xvt[:], t_xit[:])

            pad72 = pa1.tile([96, 50, 50], F32, tag="pad72")
            nc.vector.memset(pad72[:], 0.0)

            # in_proj: merged [zv|xv] / [zi|xi] matmuls + sub
            t_zacc = pa1.tile([DSL, 2, len(ROWCHUNKS)], F32, tag="zacc")
            t_zc = {"vi": pa1.tile([DSL, HW], F32, tag="zcvi", name="zcvi"),
                    "ir": pa1.tile([DSL, HW], F32, tag="zcir", name="zcir")}
            for im, (mod, w48, xt) in enumerate(
                    (("vi", t_w48v, t_xvt), ("ir", t_w48i, t_xit))):
                for ic, (r0, nr) in enumerate(ROWCHUNKS):
                    cols = slice(r0 * W, (r0 + nr) * W)
                    p48 = pap.tile([64, RCH], F32, tag="p48")
                    mmr(p48[:, :nr * W], w48[:], xt[:, cols],
                        start=True, stop=True)
                    nc.scalar.activation(t_zc[mod][:, cols],
                                         p48[32:56, :nr * W], AF.Silu,
                                         accum_out=t_zacc[:, im, ic:ic + 1])
                    o = XOFF[mod]
                    nc.scalar.copy(
                        pad72[o:o + DSL, 1 + r0:1 + r0 + nr, 1:49]
                        .bitcast(F32R),
                        p48[0:DSL, :nr * W]
                        .rearrange("p (a b) -> p a b", a=nr))
            for (r0, nr) in ROWCHUNKS:
                cols = slice(r0 * W, (r0 + nr) * W)
                p24 = pas.tile([28, RCH], F32, tag="px")
                mmr(p24[0:DSL, :nr * W], t_wsub[:], t_xdiff[:, cols],
                    start=True, stop=True)
                nc.scalar.copy(
                    pad72[0:DSL, 1 + r0:1 + r0 + nr, 1:49].bitcast(F32R),
                    p24[0:DSL, :nr * W].rearrange("p (a b) -> p a b", a=nr))

            # chan-attn pooled stats -> v1 partials -> tiny AR
            t_pool = pa1.tile([DSL, 4], F32, tag="tpool")
            nc.vector.tensor_reduce(t_pool[:, 0:1], t_zacc[:, 0, :],
                                    axis=mybir.AxisListType.X, op=OP.add)
            nc.vector.tensor_reduce(t_pool[:, 1:2], t_zc["vi"][:],
                                    axis=mybir.AxisListType.X, op=OP.max)
            nc.vector.tensor_reduce(t_pool[:, 2:3], t_zacc[:, 1, :],
                                    axis=mybir.AxisListType.X, op=OP.add)
            nc.vector.tensor_reduce(t_pool[:, 3:4], t_zc["ir"][:],
                                    axis=mybir.AxisListType.X, op=OP.max)
            t_v1 = pa1.tile([12, 4], F32, tag="tv1")
            p_v1 = pav1.tile([12, 4], F32, tag="pv1")
            for j in range(4):
                nc.tensor.matmul(p_v1[:, j:j + 1], t_f1[:, j, :],
                                 t_pool[:, j:j + 1], start=True, stop=True)
            nc.scalar.copy(t_v1[:], p_v1[:])
            nc.sync.dma_start(out=d_v1i[:], in_=t_v1[:])

            # depthwise conv 3x3: block-diag 72-channel, 9 taps
            for (r0, nr) in ROWCHUNKS:
                p_c = pav.tile([96, RCH], F32, tag="pconv")
                for tap in range(9):
                    dy, dx = tap // 3, tap % 3
                    mmr(p_c[:, :nr * W], t_w72[:, tap, :],
                        pad72[:, r0 + dy:r0 + dy + nr, dx:dx + 48],
                        start=(tap == 0), stop=(tap == 8))
                nc.scalar.activation(
                    t_xs72[:, r0 * W:(r0 + nr) * W],
                    p_c[:, :nr * W], AF.Silu, bias=t_b72[:, 0:1], scale=1.0)

            # x_dbl partials per scan tile -> DRAM -> chunked AllReduce
            ar_inst = [None, None, None]
            for tg in range(3):
                for (r0, nr) in ROWCHUNKS:
                    cols = slice(r0 * W, (r0 + nr) * W)
                    p84 = pas.tile([28, RCH], F32, tag="px")
                    nc.tensor.matmul(p84[:, :nr * W], t_w84[:, tg, :],
                                     t_xs72[:, cols], start=True, stop=True)
                    t_xe = pa.tile([28, RCH], BF16, tag="txdbl", name="t_xe")
                    nc.scalar.copy(t_xe[:, :nr * W], p84[:, :nr * W])
                    for hh in range(2):
                        nc.sync.dma_start(
                            out=d_ri[tg][hh, :, r0 * W:(r0 + nr) * W],
                            in_=t_xe[hh * 14:(hh + 1) * 14, :nr * W])
                ar_inst[tg] = nc.gpsimd.collective_compute(
                    "AllReduce", OP.add, RG,
                    ins=[d_ri[tg][:]], outs=[d_ro[tg][:]])
            ar_v1 = nc.gpsimd.collective_compute(
                "AllReduce", OP.add, RG, ins=[d_v1i[:]], outs=[d_v1o[:]])

        # =========== PHASE B: scan middle (full-tile staging) ===========
        with tc.tile_pool(name="pb", bufs=2) as pb, \
             tc.tile_pool(name="pb2", bufs=2) as pb2, \
             tc.tile_pool(name="pbp", bufs=2, space="PSUM") as pbp, \
             tc.tile_pool(name="pby", bufs=2, space="PSUM") as pby:
            for t in range(3):
                segs = ([(0, L, t)] if t < 2 else
                        [(0, HW, 2), (HW, L, 3)])  # (start, end, k) tile cols
                yc0 = HW if t < 2 else 0            # y column span
                dro = d_ro[t]
                # ---- full-tile staged loads (DMA) ----
                t_rR = pb2.tile([RNK, L], BF16, tag="rR")
                for hh in range(2):
                    ld = nc.sync.dma_start(
                        out=t_rR[:, hh * HW:(hh + 1) * HW],
                        in_=dro[hh, 0:RNK, :])
                    add_dep_helper(ld.ins, ar_inst[t].ins,
                                   reason="rR after AR")
                # B replicated to lanes (lane = n*DSL + d) via broadcast DMA
                t_Brep = pb.tile([LANES, L], BF16, tag="Brep")
                for half in (0, 1):
                    ld = nc.sync.dma_start(
                        out=t_Brep[:, half * HW:(half + 1) * HW],
                        in_=dro[half, RNK:RNK + NST, :]
                        .unsqueeze(1).broadcast_to([NST, DSL, HW]))
                    add_dep_helper(ld.ins, ar_inst[t].ins,
                                   reason="Brep after AR")
                # xs replicated to lanes (4 plain copies per half)
                t_urep = pb.tile([LANES, L], BF16, tag="urep")
                for half in (0, 1):
                    o = XOFF[TMODS[t][half]]
                    for n in range(NST):
                        nc.sync.dma_start(
                            out=t_urep[n * DSL:(n + 1) * DSL,
                                       half * HW:(half + 1) * HW],
                            in_=t_xs72[o:o + DSL, :])
                # C replicated (y cols only)
                t_Crep = pb.tile([LANES, L], BF16, tag="Crep")
                for half in ((1,) if t < 2 else (0, 1)):
                    ld = nc.sync.dma_start(
                        out=t_Crep[:, half * HW:(half + 1) * HW],
                        in_=dro[half, RNK + NST:14, :]
                        .unsqueeze(1).broadcast_to([NST, DSL, HW]))
                    add_dep_helper(ld.ins, ar_inst[t].ins,
                                   reason="Crep after AR")

                # ---- dts chunks: matmul + Exp evac (exp table) ----
                t_et = pb2.tile([LANES, L], F32, tag="eta")
                for c in range(NCH):
                    c0 = c * CH
                    pieces = [(max(s, c0), min(e, c0 + CH), k)
                              for (s, e, k) in segs
                              if e > c0 and s < c0 + CH]
                    p_dts = pbp.tile([LANES, CH], F32, tag="dts")
                    for (s, e, k) in pieces:
                        nc.tensor.matmul(p_dts[:, s - c0:e - c0],
                                         t_wdtr[:, k, :],
                                         t_rR[:, s:e], start=True, stop=True)
                    for (s, e, k) in pieces:
                        nc.scalar.activation(t_et[:, s:e],
                                             p_dts[:, s - c0:e - c0], AF.Exp,
                                             bias=t_dtb[:, k:k + 1], scale=1.0)
                # ---- delta = softplus (ln table), then a = exp(asc*delta) ----
                t_delta = pb.tile([LANES, L], BF16, tag="delta")
                nc.scalar.activation(t_delta[:], t_et[:], AF.Ln,
                                     bias=1.0, scale=1.0)
                t_a = pb2.tile([LANES, L], F32, tag="eta")  # reuse et buffer
                for (s, e, k) in segs:
                    nc.scalar.activation(t_a[:, s:e], t_delta[:, s:e], AF.Exp,
                                         bias=0.0, scale=t_asc[:, k:k + 1])
                # ---- b = delta * B_rep * xs_rep (in-place, DVE + gpsimd) ----
                SPL = 3072   # DVE gets 2/3, gpsimd 1/3 (eff ~0.42)
                nc.vector.tensor_mul(t_Brep[:, 0:SPL], t_delta[:, 0:SPL],
                                     t_Brep[:, 0:SPL])
                nc.gpsimd.tensor_mul(t_Brep[:, SPL:L], t_delta[:, SPL:L],
                                     t_Brep[:, SPL:L])
                nc.vector.tensor_mul(t_Brep[:, 0:SPL], t_Brep[:, 0:SPL],
                                     t_urep[:, 0:SPL])
                nc.gpsimd.tensor_mul(t_Brep[:, SPL:L], t_Brep[:, SPL:L],
                                     t_urep[:, SPL:L])
                # ---- scan ----
                t_h = pb.tile([LANES, L], F32, tag="h")
                if t < 2:
                    nc.vector.tensor_tensor_scan(t_h[:].bitcast(F32R), t_a[:],
                                                 t_Brep[:], 0.0,
                                                 OP.mult, OP.add)
                else:
                    for (s, e, k) in segs:   # reversed scans, fresh state
                        nc.vector.tensor_tensor_scan(
                            t_h[:, s:e][:, ::-1].bitcast(F32R),
                            t_a[:, s:e][:, ::-1],
                            t_Brep[:, s:e][:, ::-1], 0.0, OP.mult, OP.add)
                # ---- hc = h * C_rep (in-place into h), y cols only ----
                nc.gpsimd.tensor_mul(t_h[:, yc0:L].bitcast(F32R),
                                     t_h[:, yc0:L], t_Crep[:, yc0:L])
                # ---- y = m96 @ hc (+ D skip) -> accumulate into yvi/yir ----
                c0 = yc0
                while c0 < L:
                    c1 = min(c0 + CH, L)
                    p_y = pby.tile([DSL, CH], F32, tag="y")
                    mmr(p_y[:, :c1 - c0], t_m96[:], t_h[:, c0:c1],
                        start=True, stop=(t == 2))
                    if t < 2:   # D-skip, combined (D_k + D_{k+2}) on fwd tiles
                        o = XOFF[TMODS[t][1]]
                        nc.tensor.matmul(
                            p_y[:, :c1 - c0], t_diagd[o:o + DSL, t, :],
                            t_xs72[o:o + DSL, c0 - HW:c1 - HW],
                            start=False, stop=True)
                        dst = t_yvi if t == 0 else t_yir
                        nc.scalar.copy(dst[:, c0 - HW:c1 - HW],
                                       p_y[:, :c1 - c0])
                    else:
                        for (s, e) in ((c0, min(c1, HW)), (max(c0, HW), c1)):
                            if e <= s:
                                continue
                            if e <= HW:
                                nc.vector.tensor_add(
                                    t_yvi[:, s:e], t_yvi[:, s:e],
                                    p_y[:, s - c0:e - c0])
                            else:
                                nc.vector.tensor_add(
                                    t_yir[:, s - HW:e - HW],
                                    t_yir[:, s - HW:e - HW],
                                    p_y[:, s - c0:e - c0])
                    c0 = c1

        # =========== A2A: reshard y channels -> positions ===========
        for j in range(NCORES):
            nc.sync.dma_start(out=d_a2i[j, 0:DSL, :],
                              in_=t_yvi[:, j * PC:(j + 1) * PC])
            nc.sync.dma_start(out=d_a2i[j, DSL:2 * DSL, :],
                              in_=t_yir[:, j * PC:(j + 1) * PC])
        a2a_inst = nc.gpsimd.collective_compute(
            "AllToAll", OP.bypass, RG, ins=[d_a2i[:]], outs=[d_a2o[:]])
        t_v1o = big.tile([12, 4], F32, tag="v1o")
        ld = nc.sync.dma_start(out=t_v1o[:], in_=d_v1o[:])
        add_dep_helper(ld.ins, ar_v1.ins, reason="v1 after AR")

        # =========== PHASE C: LN + gate + out (position-sharded) ===========
        with tc.tile_pool(name="pcq", bufs=2) as pcq, \
             tc.tile_pool(name="pcp", bufs=1, space="PSUM") as pcp:
            # gather y chunks [96, PC] x (2 chunks, 2 mods)
            t_y = {}
            for mod, roff in (("vi", 0), ("ir", DSL)):
                for ck in range(2):
                    ty = pcq.tile([DM, PC], F32, tag=f"y{mod}{ck}",
                                  name=f"y{mod}{ck}")
                    for jj in range(4):
                        j = ck * 4 + jj
                        ld = nc.sync.dma_start(
                            out=ty[jj * DSL:(jj + 1) * DSL, :].bitcast(F32R),
                            in_=d_a2o[j, roff:roff + DSL, :].bitcast(F32R))
                        add_dep_helper(ld.ins, a2a_inst.ins,
                                       reason="y after A2A")
                    t_y[(mod, ck)] = ty
            # chan-attn scales s = 1 + sigmoid(f2 @ (relu(va)+relu(vm)))
            t_vr = pcq.tile([12, 4], F32, tag="vr")
            nc.scalar.activation(t_vr[:], t_v1o[:], AF.Relu)
            t_vw = pcq.tile([12, 2], F32, tag="vw")
            nc.vector.tensor_add(t_vw[:, 0:1], t_vr[:, 0:1], t_vr[:, 1:2])
            nc.vector.tensor_add(t_vw[:, 1:2], t_vr[:, 2:3], t_vr[:, 3:4])
            t_s = {}
            for ck in range(2):
                p_ca = pcp.tile([DM, 2], F32, tag="pca")
                for mod_i in range(2):
                    nc.tensor.matmul(p_ca[:, mod_i:mod_i + 1],
                                     t_f2[:, mod_i, ck, :],
                                     t_vw[:, mod_i:mod_i + 1],
                                     start=True, stop=True)
                t_e = pcq.tile([DM, 2], F32, tag="cae")
                nc.scalar.activation(t_e[:], p_ca[:], AF.Exp,
                                     bias=0.0, scale=-1.0)
                nc.vector.tensor_scalar_add(t_e[:], t_e[:], 1.0)
                t_r = pcq.tile([DM, 2], F32, tag=f"car{ck}", name=f"car{ck}")
                nc.vector.reciprocal(t_r[:], t_e[:])          # sigmoid
                nc.vector.tensor_scalar_add(t_r[:], t_r[:], 1.0)  # 1+sigmoid
                t_s[ck] = t_r
            # z recompute at my positions: z = x @ Wz, silu via exp+recip
            t_z = {}
            for zi, (mod, ck) in enumerate(
                    (("vi", 0), ("vi", 1), ("ir", 0), ("ir", 1))):
                xt = t_xvc if mod == "vi" else t_xic
                p_z = pcp.tile([DM, PC], F32, tag="pz2")
                mmr(p_z[:], t_wz[:, zi, :], xt[:],
                    start=True, stop=True)
                t_e = pcq.tile([DM, PC], F32, tag="ze")
                nc.scalar.activation(t_e[:], p_z[:], AF.Exp,
                                     bias=0.0, scale=-1.0)
                nc.vector.tensor_scalar_add(t_e[:], t_e[:], 1.0)
                t_r = pcq.tile([DM, PC], F32, tag="zr")
                nc.vector.reciprocal(t_r[:], t_e[:])
                tz = pcq.tile([DM, PC], F32, tag=f"z{zi}", name=f"z{zi}")
                nc.vector.tensor_mul(tz[:], p_z[:], t_r[:])
                t_z[(mod, ck)] = tz
            # LN per modality
            t_fin = {}
            for mod in ("vi", "ir"):
                p_s1 = pcp.tile([1, PC], F32, tag="s1")
                p_s2 = pcp.tile([1, PC], F32, tag="s2")
                for ck in range(2):
                    nc.tensor.matmul(p_s1[:], t_onec[:],
                                     t_y[(mod, ck)][:], start=(ck == 0),
                                     stop=(ck == 1))
                for ck in range(2):
                    t_sq = pcq.tile([DM, PC], F32, tag="sq")
                    nc.scalar.activation(t_sq[:], t_y[(mod, ck)][:], AF.Square)
                    nc.tensor.matmul(p_s2[:], t_onec[:],
                                     t_sq[:], start=(ck == 0), stop=(ck == 1))
                t_mu = pcq.tile([1, PC], F32, tag="mu")
                nc.vector.tensor_scalar_mul(t_mu[:], p_s1[:], 1.0 / DI)
                t_musq = pcq.tile([1, PC], F32, tag="musq")
                nc.vector.tensor_mul(t_musq[:], t_mu[:], t_mu[:])
                t_var = pcq.tile([1, PC], F32, tag="var")
                nc.vector.scalar_tensor_tensor(t_var[:], p_s2[:], 1.0 / DI,
                                               t_musq[:], OP.mult, OP.subtract)
                t_eps = pcq.tile([1, 1], F32, tag="eps")
                nc.vector.memset(t_eps[:], 1e-5)
                t_lnv = pcq.tile([1, PC], F32, tag="lnv")
                nc.scalar.activation(t_lnv[:], t_var[:], AF.Ln,
                                     bias=t_eps[:], scale=1.0)
                t_rstd = pcq.tile([1, PC], F32, tag="rstd")
                nc.scalar.activation(t_rstd[:], t_lnv[:], AF.Exp,
                                     bias=0.0, scale=-0.5)
                t_mur = pcq.tile([1, PC], F32, tag="mur")
                nc.vector.tensor_mul(t_mur[:], t_mu[:], t_rstd[:])
                p_q = pcp.tile([DM, PC], F32, tag="pq")
                nc.tensor.matmul(p_q[:], t_oner[:], t_rstd[:],
                                 start=True, stop=True)
                p_m = pcp.tile([DM, PC], F32, tag="pm")
                nc.tensor.matmul(p_m[:], t_oner[:], t_mur[:],
                                 start=True, stop=True)
                gb = {"vi": (0, 1), "ir": (2, 3)}[mod]
                for ck in range(2):
                    t_t = pcq.tile([DM, PC], F32, tag="lt")
                    nc.vector.tensor_mul(t_t[:], t_y[(mod, ck)][:], p_q[:])
                    t_t2 = pcq.tile([DM, PC], F32, tag="lt2")
                    nc.vector.tensor_sub(t_t2[:], t_t[:], p_m[:])
                    t_yn = pcq.tile([DM, PC], F32, tag="yn")
                    nc.scalar.activation(t_yn[:], t_t2[:], AF.Identity,
                                         bias=t_lnw[:, ck, gb[1]:gb[1] + 1],
                                         scale=t_lnw[:, ck, gb[0]:gb[0] + 1])
                    # gate: fin += yn * z * s
                    t_m1 = pcq.tile([DM, PC], F32, tag="m1")
                    nc.vector.tensor_mul(t_m1[:], t_yn[:], t_z[(mod, ck)][:])
                    if mod == "vi":
                        t_f = pcq.tile([DM, PC], F32, tag=f"fin{ck}",
                                       name=f"fin{ck}")
                        nc.vector.tensor_scalar_mul(t_f[:].bitcast(F32R),
                                                    t_m1[:],
                                                    t_s[ck][:, 0:1])
                        t_fin[ck] = t_f
                    else:
                        nc.vector.scalar_tensor_tensor(
                            t_fin[ck][:].bitcast(F32R), t_m1[:],
                            t_s[ck][:, 1:2], t_fin[ck][:],
                            OP.mult, OP.add)
            p_o = pcp.tile([DM, PC], F32, tag="po")
            for ck in range(2):
                mmr(p_o[:], t_wout[:, ck, :], t_fin[ck][:],
                    start=(ck == 0), stop=(ck == 1))
            t_o = pcq.tile([DM, PC], F32, tag="o")
            nc.scalar.copy(t_o[:], p_o[:])
            nc.sync.dma_start(out=o_out[:], in_=t_o[:])

    nc.finalize()
    return nc


def _prep_inputs(inputs):
    """Host-side prep: slice/transpose weights per core. Returns in_maps."""
    g = {k: np.asarray(v, dtype=np.float32) for k, v in inputs.items()}
    x_vi = g["x_vi"].reshape(HW, DM)
    x_ir = g["x_ir"].reshape(HW, DM)
    xvt = np.ascontiguousarray(x_vi.T)
    xit = np.ascontiguousarray(x_ir.T)
    A = -np.exp(g["A_logs"]).reshape(K, DI, NST)
    Ds = g["Ds"].reshape(K, DI)
    in_maps = []
    for c in range(NCORES):
        S = slice(c * DSL, (c + 1) * DSL)
        m = {}
        m["xvt"], m["xit"] = xvt, xit
        w48v = np.zeros((DM, 64), np.float32)
        w48v[:, 0:DSL] = g["W_vi"][S].T
        w48v[:, 32:56] = g["W_vi"][DI:][S].T
        m["w48v"] = w48v
        w48i = np.zeros((DM, 64), np.float32)
        w48i[:, 0:DSL] = g["W_ir"][S].T
        w48i[:, 32:56] = g["W_ir"][DI:][S].T
        m["w48i"] = w48i
        m["wsub"] = np.ascontiguousarray(g["W_sub"][S].T)
        w72 = np.zeros((96, 9, 96), np.float32)
        b72 = np.zeros((96, 1), np.float32)
        for nm in ("sub", "vi", "ir"):
            o = XOFF[nm]
            cw = g[f"conv_w_{nm}"][S, 0]      # [DSL, 3, 3]
            for tap in range(9):
                for d in range(DSL):
                    w72[o + d, tap, o + d] = cw[d, tap // 3, tap % 3]
            b72[o:o + DSL, 0] = g[f"conv_b_{nm}"][S]
        m["w72"], m["b72"] = w72, b72
        # x_dbl lhsT per scan tile: blocks (tile, half) -> (k, src mod)
        w84 = np.zeros((96, 3, 28), np.float32)
        BLK = (((0, "sub"), (0, "vi")), ((1, "sub"), (1, "ir")),
               ((2, "vi"), (3, "ir")))
        for tg in range(3):
            for half, (k, nm) in enumerate(BLK[tg]):
                o = XOFF[nm]
                w84[o:o + DSL, tg, half * 14:(half + 1) * 14] = \
                    g["x_proj_weight"][k][:, S].T
        m["w84"] = w84.astype(ml_dtypes.bfloat16)
        wdtr = np.zeros((RNK, K, LANES), np.float32)
        dtb = np.zeros((LANES, K), np.float32)
        asc = np.zeros((LANES, K), np.float32)
        for k in range(K):
            for n in range(NST):
                for d in range(DSL):
                    lane = n * DSL + d
                    wdtr[:, k, lane] = g["dt_projs_weight"][k, c * DSL + d, :]
                    dtb[lane, k] = g["dt_projs_bias"][k, c * DSL + d]
                    asc[lane, k] = A[k, c * DSL + d, n]
        m["wdtr"] = wdtr.astype(ml_dtypes.bfloat16)
        m["dtb"], m["asc"] = dtb, asc
        m96 = np.zeros((LANES, DSL), np.float32)
        for n in range(NST):
            for d in range(DSL):
                m96[n * DSL + d, d] = 1
        m["m96"] = m96
        diagd = np.zeros((96, 2, DSL), np.float32)
        np.fill_diagonal(diagd[XOFF["vi"]:XOFF["vi"] + DSL, 0, :],
                         Ds[0, S] + Ds[2, S])
        np.fill_diagonal(diagd[XOFF["ir"]:XOFF["ir"] + DSL, 1, :],
                         Ds[1, S] + Ds[3, S])
        m["diagd"] = diagd.astype(ml_dtypes.bfloat16)
        f1 = np.zeros((DSL, 4, 12), np.float32)
        f1[:, 0] = g["ca_vi_f1"][:, S].T / HW
        f1[:, 1] = g["ca_vi_f1"][:, S].T
        f1[:, 2] = g["ca_ir_f1"][:, S].T / HW
        f1[:, 3] = g["ca_ir_f1"][:, S].T
        m["f1"] = f1
        f2 = np.zeros((12, 2, 2, DM), np.float32)
        for ck in range(2):
            f2[:, 0, ck] = g["ca_vi_f2"][ck * DM:(ck + 1) * DM].T
            f2[:, 1, ck] = g["ca_ir_f2"][ck * DM:(ck + 1) * DM].T
        m["f2"] = f2
        lnw = np.zeros((DM, 2, 4), np.float32)
        for ck in range(2):
            cs = slice(ck * DM, (ck + 1) * DM)
            lnw[:, ck, 0] = g["ln_vi_g"][cs]
            lnw[:, ck, 1] = g["ln_vi_b"][cs]
            lnw[:, ck, 2] = g["ln_ir_g"][cs]
            lnw[:, ck, 3] = g["ln_ir_b"][cs]
        m["lnw"] = lnw
        wout = np.zeros((DM, 2, DM), np.float32)
        for ck in range(2):
            wout[:, ck] = g["W_out"][:, ck * DM:(ck + 1) * DM].T
        m["wout"] = wout
        wz = np.zeros((DM, 4, DM), np.float32)
        wz[:, 0] = g["W_vi"][DI:][0:DM].T
        wz[:, 1] = g["W_vi"][DI:][DM:DI].T
        wz[:, 2] = g["W_ir"][DI:][0:DM].T
        wz[:, 3] = g["W_ir"][DI:][DM:DI].T
        m["wz"] = wz
        m["onec"] = np.ones((DM, 1), np.float32)
        m["oner"] = np.ones((1, DM), np.float32)
        m["xvc"] = np.ascontiguousarray(xvt[:, c * PC:(c + 1) * PC])
        m["xic"] = np.ascontiguousarray(xit[:, c * PC:(c + 1) * PC])
        in_maps.append(m)
    return in_maps


def kernel(**inputs):
    if "nc" not in _cache:
        _cache["nc"] = _build()
    nc = _cache["nc"]
    in_maps = _prep_inputs(inputs)
    res = run_bass_kernel_spmd(nc, in_maps, core_ids=list(range(NCORES)))
    out = np.zeros((DM, HW), np.float32)
    for c in range(NCORES):
        out[:, c * PC:(c + 1) * PC] = res.results[c]["out"]
    return out.T.reshape(B, H, W, DM).astype(np.float32)
